# revision 8
# baseline (speedup 1.0000x reference)
"""Trainium2 Bass kernel: causal multi-head attention with RoPE.

Model: B=4, L=2048, H=2048, NH=16 heads, head_dim=128.
  q = x @ Wq.T ; k = x @ Wk.T ; v = x @ Wv.T        (per-head split)
  q, k <- RoPE(q, k)
  attn = softmax(mask(q k^T / sqrt(hd)))
  out  = (attn @ v) heads-concat @ Wo.T

Sharding (8 cores): hybrid batch x tensor-parallel.  Core c handles
batch b = c//2 and heads half*8..half*8+7 with half = c%2.  Wq/Wk/Wv are
column-sharded (8 heads per core), Wo row-sharded; each core produces a
partial y[b] (bf16) and the host sums the two partials per batch.

Per-core dataflow (SBUF-resident, bf16 inputs / fp32 accumulation):
  phase A: QK pass per 512-pos x chunk (x loaded once for both):
           hc-outer accumulation into 8 PSUM banks so the first matmul
           needs only 1.5MB of DMA; fused RoPE (partition-swap DMA +
           DVE).  Then a V pass (pos-major, 8 banks) with PSUM->SBUF
           copies on DVE so ACT is drained before attention.
  phase B: flash-style causal attention per (head, 512-wide q chunk):
           S^T tiles into a 5-bank PSUM ring, exp on ACT over multi-bank
           group spans (group sizes DP-chosen to trade ACT instruction
           overhead vs dead-margin columns), triangular-mask multiply on
           diagonal blocks (DVE), O^T accumulation (PE), softmax
           denominator via fp8 DoubleRow ones-matmul over pair-packed
           fp8 copies of P (DVE converts), per-head batched
           normalization (unnormalized O commutes with the denominator).
  phase C: y^T partial = Wo_shard O^T (PE) -> DRAM bf16.
"""

import math
import numpy as np

B, L, H, NH, HD = 4, 2048, 2048, 16, 128
ROPE_BASE = 10000.0
NCORES = 8
HPC = 8          # heads per core
QC = 512         # q chunk width
NQC = L // QC    # 4 q chunks
NKB = L // 128   # 16 kp blocks
SCALE = 1.0 / math.sqrt(HD)
NHC = H // 128   # 16 input-feature blocks

_cache = {}


def _analyze_mask(mask2d):
    """Classify each (q_block, kp_block) 128x128 block of the [L, L] mask."""
    nb = L // 128
    kind = [[0] * nb for _ in range(nb)]
    patterns = []
    pat_key_to_idx = {}
    block_pat = {}
    for qb in range(nb):
        rows = mask2d[qb * 128:(qb + 1) * 128]
        for kb in range(nb):
            blk = rows[:, kb * 128:(kb + 1) * 128]
            s = int(blk.sum())
            if s == 0:
                kind[qb][kb] = 0
            elif s == 128 * 128:
                kind[qb][kb] = 1
            else:
                kind[qb][kb] = 2
                key = blk.tobytes()
                idx = pat_key_to_idx.get(key)
                if idx is None:
                    idx = len(patterns)
                    pat_key_to_idx[key] = idx
                    # stored transposed: S^T tiles are [kp, q]
                    patterns.append(np.ascontiguousarray(blk.T))
                block_pat[(qb, kb)] = idx
    return kind, patterns, block_pat


def _chunk_plan(kind, block_pat):
    """Per q-chunk block list: (i, w0, [(t, pat), ...]) for live kp blocks.

    w0 = first live 128-col offset within the chunk; requires the causal
    staircase (w0 nondecreasing in i, first block full, even count)."""
    plans = []
    for j in range(NQC):
        blocks = []
        for i in range(NKB):
            live = [t for t in range(4) if kind[4 * j + t][i] != 0]
            if not live:
                continue
            w0 = live[0] * 128
            assert live == list(range(live[0], 4)), "non-staircase mask"
            diags = [(t, block_pat[(4 * j + t, i)]) for t in live
                     if kind[4 * j + t][i] == 2]
            blocks.append((i, w0, diags))
        assert blocks and blocks[0][1] == 0, "first live block must be full"
        assert len(blocks) % 2 == 0, "need even live-block count per chunk"
        for a, b in zip(blocks, blocks[1:]):
            assert a[1] <= b[1], "w0 must be nondecreasing"
        plans.append(blocks)
    return plans


def _plan_groups(blocks, parity0):
    """Split a chunk's blocks into exp groups for the 5-slot PSUM ring.

    Ring = A slots (0,1) + B slots (2,3,4), strictly alternating; a group
    may underfill.  DP minimizes ACT cost = sum(OH + len*512 - w0_first).
    Returns (groups, parity_out), groups = lists of indices into blocks."""
    OH = 390
    n = len(blocks)
    INF = float("inf")
    dp = [[None, None] for _ in range(n + 1)]
    dp[n] = [(0, 0), (0, 0)]
    for i in range(n - 1, -1, -1):
        for p in (0, 1):
            cap = 2 if p == 0 else 3
            best = (INF, 0)
            for ln in range(1, min(cap, n - i) + 1):
                cost = OH + ln * 512 - blocks[i][1] + dp[i + ln][1 - p][0]
                if cost < best[0]:
                    best = (cost, ln)
            dp[i][p] = best
    groups = []
    i, p = 0, parity0
    while i < n:
        ln = dp[i][p][1]
        groups.append(list(range(i, i + ln)))
        i += ln
        p = 1 - p
    return groups, p


def _build(kind, block_pat, n_patterns):
    """Build the SPMD bass program (same for all 8 cores)."""
    import concourse.bacc as bacc
    import concourse.mybir as mybir
    import concourse.tile as tile
    from concourse.tile import add_dep_helper

    fp32 = mybir.dt.float32
    bf16 = mybir.dt.bfloat16
    fp8 = mybir.dt.float8e4
    EXP = mybir.ActivationFunctionType.Exp
    DR = mybir.MatmulPerfMode.DoubleRow

    nc = bacc.Bacc("TRN2", target_bir_lowering=False, debug=False)

    xT = nc.dram_tensor("xT", [H, L], bf16, kind="ExternalInput")
    wqT = nc.dram_tensor("wqT", [H, HPC * HD], bf16, kind="ExternalInput")
    wkT = nc.dram_tensor("wkT", [H, HPC * HD], bf16, kind="ExternalInput")
    wvT = nc.dram_tensor("wvT", [H, HPC * HD], bf16, kind="ExternalInput")
    woT = nc.dram_tensor("woT", [HPC * HD, H], bf16, kind="ExternalInput")
    cosd = nc.dram_tensor("cosd", [HD, L], bf16, kind="ExternalInput")
    sinmd = nc.dram_tensor("sinmd", [HD, L], bf16, kind="ExternalInput")
    npat = max(n_patterns, 1)
    maskd = nc.dram_tensor("maskd", [npat, 128, 128], bf16, kind="ExternalInput")
    yT = nc.dram_tensor("yT", [H, L], bf16, kind="ExternalOutput")

    plans = _chunk_plan(kind, block_pat)

    with tile.TileContext(nc) as tc:
        with tc.tile_pool(name="persist", bufs=1, side="left") as persist:
            cst = persist.tile([128, npat * 128], bf16, tag="cst")
            ones8 = persist.tile([128, 2, 16], fp8, tag="ones8")
            QTa = persist.tile([HD, HPC, L], bf16, tag="qta")
            KTa = persist.tile([HD, HPC, L], bf16, tag="kta")
            Va = persist.tile([128, NKB, HPC * HD], bf16, tag="va")

            # ---------------- phase A ----------------
            wpool_cm = tc.tile_pool(name="wpool", bufs=2, side="right")
            wpool = wpool_cm.__enter__()
            ropec_cm = tc.tile_pool(name="ropec", bufs=1, side="right")
            ropec = ropec_cm.__enter__()
            xp_cm = tc.tile_pool(name="xp", bufs=2, side="right")
            xp = xp_cm.__enter__()
            tp_cm = tc.tile_pool(name="tpool", bufs=2, side="right")
            tp = tp_cm.__enter__()
            psA_cm = tc.tile_pool(name="psA", bufs=8, space="PSUM")
            psA = psA_cm.__enter__()

            wq_sb = wpool.tile([128, NHC, HPC * HD], bf16, tag="w", name="w_q")
            wk_sb = wpool.tile([128, NHC, HPC * HD], bf16, tag="w", name="w_k")
            cos_sb = ropec.tile([HD, L], bf16, tag="cos")
            sinm_sb = ropec.tile([HD, L], bf16, tag="sinm")

            wr = {"q": wqT[:].rearrange("(a p) m -> p a m", p=128),
                  "k": wkT[:].rearrange("(a p) m -> p a m", p=128),
                  "v": wvT[:].rearrange("(a p) m -> p a m", p=128)}

            # startup: interleave x(j0) and wq groups; defer the rest
            x0_sb = xp.tile([128, NHC, QC], bf16, tag="x", name="x0")
            xr0 = xT[:, 0:QC].rearrange("(a p) m -> p a m", p=128)
            x0_dmas, wq_dmas = [], []
            for g in range(4):
                wq_dmas.append(nc.sync.dma_start(
                    out=wq_sb[:, 4 * g:4 * g + 4, :],
                    in_=wr["q"][:, 4 * g:4 * g + 4, :]))
                x0_dmas.append(nc.sync.dma_start(
                    out=x0_sb[:, 4 * g:4 * g + 4, :],
                    in_=xr0[:, 4 * g:4 * g + 4, :]))
            nc.gpsimd.dma_start(out=cos_sb[:], in_=cosd[:])
            nc.gpsimd.dma_start(out=sinm_sb[:], in_=sinmd[:])
            for p in range(n_patterns):
                nc.gpsimd.dma_start(out=cst[:, p * 128:(p + 1) * 128],
                                    in_=maskd[p])
            nc.vector.memset(ones8[:], 1.0)
            for g in range(4):
                di = nc.sync.dma_start(
                    out=wk_sb[:, 4 * g:4 * g + 4, :],
                    in_=wr["k"][:, 4 * g:4 * g + 4, :])
                for d0 in (x0_dmas[3], wq_dmas[3]):
                    add_dep_helper(di.ins, d0.ins, reason="defer wk")

            def rope(out_a, h, js):
                q = out_a[:, h, js]
                rq = tp.tile([128, QC], bf16, tag="rot")
                nc.sync.dma_start(out=rq[0:64, :], in_=out_a[64:128, h, js])
                nc.sync.dma_start(out=rq[64:128, :], in_=out_a[0:64, h, js])
                nc.vector.tensor_mul(rq[:], rq[:], sinm_sb[:, js])
                nc.vector.tensor_mul(q, q, cos_sb[:, js])
                nc.vector.tensor_add(q, q, rq[:])

            # QK pass: x chunk loaded once, Q then K, hc-outer, 8 banks
            for j in range(NQC):
                js = slice(j * QC, (j + 1) * QC)
                if j == 0:
                    x_sb = x0_sb
                else:
                    x_sb = xp.tile([128, NHC, QC], bf16, tag="x", name=f"x{j}")
                    xr = xT[:, js].rearrange("(a p) m -> p a m", p=128)
                    for g in range(4):
                        nc.sync.dma_start(out=x_sb[:, 4 * g:4 * g + 4, :],
                                          in_=xr[:, 4 * g:4 * g + 4, :])
                for w_sb, out_a, tag in ((wq_sb, QTa, "q"), (wk_sb, KTa, "k")):
                    ps = [psA.tile([128, QC], fp32, tag="psA",
                                   name=f"ps{tag}{j}_{h}") for h in range(HPC)]
                    for hc in range(NHC):
                        for h in range(HPC):
                            nc.tensor.matmul(
                                ps[h][:],
                                w_sb[:, hc, h * HD:(h + 1) * HD],
                                x_sb[:, hc, :],
                                start=(hc == 0), stop=(hc == NHC - 1))
                    for h in range(HPC):
                        nc.scalar.copy(out_a[:, h, js], ps[h][:])
                        rope(out_a, h, js)

            # V pass: re-read x, pos-major, 8 banks, DVE copies
            wv_sb = wpool.tile([128, NHC, HPC * HD], bf16, tag="w", name="w_v")
            nc.sync.dma_start(out=wv_sb[:], in_=wr["v"][:])
            for j in range(NQC):
                js = slice(j * QC, (j + 1) * QC)
                x_sb = xp.tile([128, NHC, QC], bf16, tag="x", name=f"xv{j}")
                xr = xT[:, js].rearrange("(a p) m -> p a m", p=128)
                for g in range(4):
                    nc.sync.dma_start(out=x_sb[:, 4 * g:4 * g + 4, :],
                                      in_=xr[:, 4 * g:4 * g + 4, :])
                psd = [[psA.tile([128, QC], fp32, tag="psA",
                                 name=f"psv{j}_{pb}_{dc}") for dc in range(2)]
                       for pb in range(4)]
                for hc in range(NHC):
                    for pb in range(4):
                        for dc in range(2):
                            nc.tensor.matmul(
                                psd[pb][dc][:],
                                x_sb[:, hc, pb * 128:(pb + 1) * 128],
                                wv_sb[:, hc, dc * QC:(dc + 1) * QC],
                                start=(hc == 0), stop=(hc == NHC - 1))
                for pb in range(4):
                    for dc in range(2):
                        nc.vector.tensor_copy(
                            Va[:, j * 4 + pb, dc * QC:(dc + 1) * QC],
                            psd[pb][dc][:])

            tp_cm.__exit__(None, None, None)
            xp_cm.__exit__(None, None, None)
            ropec_cm.__exit__(None, None, None)
            wpool_cm.__exit__(None, None, None)
            psA_cm.__exit__(None, None, None)

            # ---------------- phases B + C ----------------
            with tc.tile_pool(name="post", bufs=1, side="left") as post:
                OTa = post.tile([HD, HPC, L], bf16, tag="ota")
                wo_sb = post.tile([128, HPC, H], bf16, tag="wo")
                nc.sync.dma_start(
                    out=wo_sb[:],
                    in_=woT[:].rearrange("(a p) m -> p a m", p=128))

                _attention(tc, nc, plans, QTa, KTa, Va, OTa, cst, ones8,
                           fp32, bf16, fp8, EXP, DR)

                with tc.tile_pool(name="ysb", bufs=3, side="right") as ypool, \
                     tc.tile_pool(name="ps_c", bufs=4, space="PSUM") as ps_c:
                    for j in range(NQC):
                        for oc in range(H // 128):
                            ps = ps_c.tile([128, QC], fp32, tag="psc")
                            for fc in range(HPC):
                                nc.tensor.matmul(
                                    ps[:],
                                    wo_sb[:, fc, oc * 128:(oc + 1) * 128],
                                    OTa[:, fc, j * QC:(j + 1) * QC],
                                    start=(fc == 0), stop=(fc == HPC - 1))
                            y_sb = ypool.tile([128, QC], bf16, tag="y")
                            nc.vector.tensor_copy(y_sb[:], ps[:])
                            nc.sync.dma_start(
                                out=yT[oc * 128:(oc + 1) * 128,
                                       j * QC:(j + 1) * QC],
                                in_=y_sb[:])

    nc.compile()
    return nc


def _attention(tc, nc, plans, QTa, KTa, Va, OTa, cst, ones8,
               fp32, bf16, fp8, EXP, DR):
    """S ring -> grouped exp -> mask -> fp8 copy -> O + DR rowsums ->
    deferred per-head normalization."""
    with tc.tile_pool(name="pring", bufs=1, side="right") as prp, \
         tc.tile_pool(name="bcp", bufs=2, side="right") as bcp, \
         tc.tile_pool(name="rrp", bufs=4, side="right") as rrp, \
         tc.tile_pool(name="ps_s", bufs=1, space="PSUM") as ps_s, \
         tc.tile_pool(name="ps_o", bufs=1, space="PSUM") as ps_o, \
         tc.tile_pool(name="ps_r", bufs=1, space="PSUM") as ps_r:
        pss = ps_s.tile([128, 5 * QC], fp32, tag="pss")   # 5-bank S ring
        pso = ps_o.tile([128, QC], fp32, tag="pso")       # O accumulator
        psr = [ps_r.tile([1, QC], fp32, tag=f"psr{i}", name=f"psr{i}")
               for i in range(2)]                         # rowsums, 2 banks
        P = prp.tile([128, 4, 3, QC], bf16, tag="P")      # 4 group-slots x 3
        ones_b = prp.tile([128, 1], bf16, tag="onesb")
        nc.vector.memset(ones_b[:], 1.0)
        Pf = P[:].rearrange("p g a n -> p (g a n)")

        def emit_group(it):
            h, j, blocks, grp, base, gs = (it["h"], it["j"], it["blocks"],
                                           it["grp"], it["base"], it["gs"])
            w0f = blocks[grp[0]][1]
            for gi, bi in enumerate(grp):
                i, w0, _ = blocks[bi]
                slot = base + gi
                lo = w0f if gi == 0 else 0
                nc.tensor.matmul(
                    pss[:, slot * QC + lo:(slot + 1) * QC],
                    KTa[:, h, i * 128:(i + 1) * 128],
                    QTa[:, h, j * QC + lo:j * QC + QC],
                    start=True, stop=True)
            span0 = base * QC + w0f
            span1 = (base + len(grp)) * QC
            po = gs * 3 * QC
            nc.scalar.activation(Pf[:, po + w0f:po + len(grp) * QC],
                                 pss[:, span0:span1], EXP, scale=SCALE)

        def consume_group(it):
            h, j, blocks, grp, gs = (it["h"], it["j"], it["blocks"],
                                     it["grp"], it["gs"])
            n = len(blocks)
            for gi, bi in enumerate(grp):
                i, w0, diags = blocks[bi]
                for t, pat in diags:
                    nc.vector.tensor_mul(
                        P[:, gs, gi, t * 128:(t + 1) * 128],
                        P[:, gs, gi, t * 128:(t + 1) * 128],
                        cst[:, pat * 128:(pat + 1) * 128])
                nc.tensor.matmul(
                    pso[:, w0:QC],
                    Va[:, i, h * HD:(h + 1) * HD],
                    P[:, gs, gi, w0:QC],
                    start=(bi == 0), stop=(bi == n - 1))
                nc.tensor.matmul(
                    psr[j % 2][0:1, w0:QC],
                    ones_b[:, 0:1],
                    P[:, gs, gi, w0:QC],
                    start=(bi == 0), stop=(bi == n - 1))
            if grp[-1] == n - 1:
                # chunk epilogue: stash O, reciprocal of the denominator
                js = slice(j * QC, (j + 1) * QC)
                nc.vector.tensor_copy(OTa[:, h, js], pso[:])
                r_sb = rrp.tile([1, QC], fp32, tag="r", name=f"r{h}_{j}")
                nc.vector.reciprocal_approx_fast(out=r_sb[0:1, :],
                                                 in_=psr[j % 2][0:1, :])
                rb_sb = rrp.tile([1, QC], bf16, tag="rb", name=f"rb{h}_{j}")
                nc.vector.tensor_copy(rb_sb[0:1, :], r_sb[0:1, :])
                nc.gpsimd.partition_broadcast(it["bc4"][:, j, :],
                                              rb_sb[0:1, :])
                if j == NQC - 1:
                    # head epilogue: batched normalization
                    nc.vector.tensor_mul(
                        OTa[:, h, :], OTa[:, h, :],
                        it["bc4"][:].rearrange("p a n -> p (a n)"))

        # flat group stream with two-group lookahead
        parity = 0
        gctr = 0
        pending = []
        for h in range(HPC):
            bc4 = bcp.tile([128, NQC, QC], bf16, tag="bc4", name=f"bc{h}")
            for j in range(NQC):
                blocks = plans[j]
                groups, parity2 = _plan_groups(blocks, parity)
                for grp in groups:
                    it = {"h": h, "j": j, "blocks": blocks, "grp": grp,
                          "base": (0, 2)[parity], "gs": gctr % 4, "bc4": bc4}
                    parity = 1 - parity
                    gctr += 1
                    emit_group(it)
                    pending.append(it)
                    if len(pending) > 2:
                        consume_group(pending.pop(0))
                assert parity == parity2
        for it in pending:
            consume_group(it)


def _prep_inputs(x, Wq, Wk, Wv, Wo, patterns):
    import ml_dtypes
    bf16 = ml_dtypes.bfloat16

    inv_freq = 1.0 / (ROPE_BASE ** (np.arange(0, HD, 2, dtype=np.float64)
                                    / HD))
    t = np.arange(L, dtype=np.float64)
    freqs = np.outer(t, inv_freq)
    emb = np.concatenate((freqs, freqs), axis=-1)
    cos = np.cos(emb).T.astype(np.float32)
    sin = np.sin(emb).T.astype(np.float32)
    sinm = sin.copy()
    sinm[0:64] = -sin[0:64]
    cos_b = cos.astype(bf16)
    sinm_b = sinm.astype(bf16)

    npat = max(len(patterns), 1)
    maskd = np.zeros((npat, 128, 128), dtype=bf16)
    for i, p in enumerate(patterns):
        maskd[i] = p.astype(np.float32).astype(bf16)

    in_maps = []
    for c in range(NCORES):
        b, half = c // 2, c % 2
        rows = slice(half * HPC * HD, (half + 1) * HPC * HD)
        in_maps.append({
            "xT": np.ascontiguousarray(x[b].T).astype(bf16),
            "wqT": np.ascontiguousarray(Wq[rows, :].T).astype(bf16),
            "wkT": np.ascontiguousarray(Wk[rows, :].T).astype(bf16),
            "wvT": np.ascontiguousarray(Wv[rows, :].T).astype(bf16),
            "woT": np.ascontiguousarray(Wo[:, rows].T).astype(bf16),
            "cosd": cos_b,
            "sinmd": sinm_b,
            "maskd": maskd,
        })
    return in_maps


def kernel(x, mask, Wq, Wk, Wv, Wo, _trace=False):
    from concourse.bass_utils import run_bass_kernel_spmd

    x = np.asarray(x, dtype=np.float32)
    mask2d = np.asarray(mask, dtype=np.int32).reshape(L, L)
    key = mask2d.tobytes()
    if key not in _cache:
        kind, patterns, block_pat = _analyze_mask(mask2d)
        nc = _build(kind, block_pat, len(patterns))
        _cache[key] = (nc, patterns)
    nc, patterns = _cache[key]

    in_maps = _prep_inputs(x, np.asarray(Wq, np.float32),
                           np.asarray(Wk, np.float32),
                           np.asarray(Wv, np.float32),
                           np.asarray(Wo, np.float32), patterns)
    res = run_bass_kernel_spmd(nc, in_maps, list(range(NCORES)),
                               trace=_trace)
    y = np.empty((B, L, H), dtype=np.float32)
    for b in range(B):
        acc = res.results[2 * b]["yT"].astype(np.float32) + \
              res.results[2 * b + 1]["yT"].astype(np.float32)
        y[b] = acc.T
    if _trace:
        kernel.last_results = res
    return y


if __name__ == "__main__":
    import reference
    inputs = reference.setup_inputs()
    inputs = {k: np.asarray(v) for k, v in inputs.items()}
    out = kernel(**inputs)
    exp = np.asarray(reference.reference(**{k: v for k, v in inputs.items()}))
    err = np.abs(out - exp).max() / np.abs(exp).max()
    print("rel err (absmax):", err)


# revision 10
# speedup vs baseline: 1.0287x; 1.0287x over previous
"""Trainium2 Bass kernel: causal multi-head attention with RoPE.

Model: B=4, L=2048, H=2048, NH=16 heads, head_dim=128.
  q = x @ Wq.T ; k = x @ Wk.T ; v = x @ Wv.T        (per-head split)
  q, k <- RoPE(q, k)
  attn = softmax(mask(q k^T / sqrt(hd)))
  out  = (attn @ v) heads-concat @ Wo.T

Sharding (8 cores): hybrid batch x tensor-parallel.  Core c handles
batch b = c//2 and heads half*8..half*8+7 with half = c%2.  Wq/Wk/Wv are
column-sharded (8 heads per core), Wo row-sharded; each core produces a
partial y[b] (bf16) and the host sums the two partials per batch.

Per-core dataflow (SBUF-resident, bf16 inputs / fp32 accumulation):
  phase A: QK pass per 512-pos x chunk (x loaded once for both):
           hc-outer accumulation into 8 PSUM banks so the first matmul
           needs only 1.5MB of DMA; fused RoPE (partition-swap DMA +
           DVE).  Then a V pass (pos-major, 8 banks) with PSUM->SBUF
           copies on DVE so ACT is drained before attention.
  phase B: flash-style causal attention per (head, 512-wide q chunk):
           S^T tiles into a 5-bank PSUM ring, exp on ACT over multi-bank
           group spans (group sizes DP-chosen to trade ACT instruction
           overhead vs dead-margin columns), triangular-mask multiply on
           diagonal blocks (DVE), O^T accumulation (PE), softmax
           denominator via fp8 DoubleRow ones-matmul over pair-packed
           fp8 copies of P (DVE converts), per-head batched
           normalization (unnormalized O commutes with the denominator).
  phase C: y^T partial = Wo_shard O^T (PE) -> DRAM bf16.
"""

import math
import numpy as np

B, L, H, NH, HD = 4, 2048, 2048, 16, 128
ROPE_BASE = 10000.0
NCORES = 8
HPC = 8          # heads per core
QC = 512         # q chunk width
NQC = L // QC    # 4 q chunks
NKB = L // 128   # 16 kp blocks
SCALE = 1.0 / math.sqrt(HD)
NHC = H // 128   # 16 input-feature blocks

_cache = {}


def _analyze_mask(mask2d):
    """Classify each (q_block, kp_block) 128x128 block of the [L, L] mask."""
    nb = L // 128
    kind = [[0] * nb for _ in range(nb)]
    patterns = []
    pat_key_to_idx = {}
    block_pat = {}
    for qb in range(nb):
        rows = mask2d[qb * 128:(qb + 1) * 128]
        for kb in range(nb):
            blk = rows[:, kb * 128:(kb + 1) * 128]
            s = int(blk.sum())
            if s == 0:
                kind[qb][kb] = 0
            elif s == 128 * 128:
                kind[qb][kb] = 1
            else:
                kind[qb][kb] = 2
                key = blk.tobytes()
                idx = pat_key_to_idx.get(key)
                if idx is None:
                    idx = len(patterns)
                    pat_key_to_idx[key] = idx
                    # stored transposed: S^T tiles are [kp, q]
                    patterns.append(np.ascontiguousarray(blk.T))
                block_pat[(qb, kb)] = idx
    return kind, patterns, block_pat


def _chunk_plan(kind, block_pat):
    """Per q-chunk block list: (i, w0, [(t, pat), ...]) for live kp blocks.

    w0 = first live 128-col offset within the chunk; requires the causal
    staircase (w0 nondecreasing in i, first block full, even count)."""
    plans = []
    for j in range(NQC):
        blocks = []
        for i in range(NKB):
            live = [t for t in range(4) if kind[4 * j + t][i] != 0]
            if not live:
                continue
            w0 = live[0] * 128
            assert live == list(range(live[0], 4)), "non-staircase mask"
            diags = [(t, block_pat[(4 * j + t, i)]) for t in live
                     if kind[4 * j + t][i] == 2]
            blocks.append((i, w0, diags))
        assert blocks and blocks[0][1] == 0, "first live block must be full"
        assert len(blocks) % 2 == 0, "need even live-block count per chunk"
        for a, b in zip(blocks, blocks[1:]):
            assert a[1] <= b[1], "w0 must be nondecreasing"
        plans.append(blocks)
    return plans


def _plan_groups(blocks, parity0):
    """Split a chunk's blocks into exp groups for the 5-slot PSUM ring.

    Ring = A slots (0,1) + B slots (2,3,4), strictly alternating; a group
    may underfill.  DP minimizes ACT cost = sum(OH + len*512 - w0_first).
    Returns (groups, parity_out), groups = lists of indices into blocks."""
    OH = 390
    n = len(blocks)
    INF = float("inf")
    dp = [[None, None] for _ in range(n + 1)]
    dp[n] = [(0, 0), (0, 0)]
    for i in range(n - 1, -1, -1):
        for p in (0, 1):
            cap = 2
            best = (INF, 0)
            for ln in range(1, min(cap, n - i) + 1):
                cost = OH + ln * 512 - blocks[i][1] + dp[i + ln][1 - p][0]
                if cost < best[0]:
                    best = (cost, ln)
            dp[i][p] = best
    groups = []
    i, p = 0, parity0
    while i < n:
        ln = dp[i][p][1]
        groups.append(list(range(i, i + ln)))
        i += ln
        p = 1 - p
    return groups, p


def _build(kind, block_pat, n_patterns):
    """Build the SPMD bass program (same for all 8 cores)."""
    import concourse.bacc as bacc
    import concourse.mybir as mybir
    import concourse.tile as tile
    from concourse.tile import add_dep_helper

    fp32 = mybir.dt.float32
    bf16 = mybir.dt.bfloat16
    fp8 = mybir.dt.float8e4
    EXP = mybir.ActivationFunctionType.Exp
    DR = mybir.MatmulPerfMode.DoubleRow

    nc = bacc.Bacc("TRN2", target_bir_lowering=False, debug=False)

    xT = nc.dram_tensor("xT", [H, L], bf16, kind="ExternalInput")
    wqT = nc.dram_tensor("wqT", [H, HPC * HD], bf16, kind="ExternalInput")
    wkT = nc.dram_tensor("wkT", [H, HPC * HD], bf16, kind="ExternalInput")
    wvT = nc.dram_tensor("wvT", [H, HPC * HD], bf16, kind="ExternalInput")
    woT = nc.dram_tensor("woT", [HPC * HD, H], bf16, kind="ExternalInput")
    cosd = nc.dram_tensor("cosd", [HD, L], bf16, kind="ExternalInput")
    sinmd = nc.dram_tensor("sinmd", [HD, L], bf16, kind="ExternalInput")
    npat = max(n_patterns, 1)
    maskd = nc.dram_tensor("maskd", [npat, 128, 128], bf16, kind="ExternalInput")
    yT = nc.dram_tensor("yT", [H, L], bf16, kind="ExternalOutput")

    plans = _chunk_plan(kind, block_pat)

    with tile.TileContext(nc) as tc:
        with tc.tile_pool(name="persist", bufs=1, side="left") as persist:
            cst = persist.tile([128, npat * 128], bf16, tag="cst")
            ones8 = persist.tile([128, 2, 16], fp8, tag="ones8")
            QTa = persist.tile([HD, HPC, L], bf16, tag="qta")
            KTa = persist.tile([HD, HPC, L], bf16, tag="kta")
            Va = persist.tile([128, NKB, HPC * HD], bf16, tag="va")

            # ---------------- phase A ----------------
            wpool_cm = tc.tile_pool(name="wpool", bufs=2, side="right")
            wpool = wpool_cm.__enter__()
            ropec_cm = tc.tile_pool(name="ropec", bufs=1, side="right")
            ropec = ropec_cm.__enter__()
            xp_cm = tc.tile_pool(name="xp", bufs=2, side="right")
            xp = xp_cm.__enter__()
            tp_cm = tc.tile_pool(name="tpool", bufs=2, side="right")
            tp = tp_cm.__enter__()
            psA_cm = tc.tile_pool(name="psA", bufs=8, space="PSUM")
            psA = psA_cm.__enter__()

            wq_sb = wpool.tile([128, NHC, HPC * HD], bf16, tag="w", name="w_q")
            wk_sb = wpool.tile([128, NHC, HPC * HD], bf16, tag="w", name="w_k")
            cos_sb = ropec.tile([HD, L], bf16, tag="cos")
            sinm_sb = ropec.tile([HD, L], bf16, tag="sinm")

            wr = {"q": wqT[:].rearrange("(a p) m -> p a m", p=128),
                  "k": wkT[:].rearrange("(a p) m -> p a m", p=128),
                  "v": wvT[:].rearrange("(a p) m -> p a m", p=128)}

            # startup: interleave x(j0) and wq groups; defer the rest
            x0_sb = xp.tile([128, NHC, QC], bf16, tag="x", name="x0")
            xr0 = xT[:, 0:QC].rearrange("(a p) m -> p a m", p=128)
            x0_dmas, wq_dmas = [], []
            for g in range(4):
                wq_dmas.append(nc.sync.dma_start(
                    out=wq_sb[:, 4 * g:4 * g + 4, :],
                    in_=wr["q"][:, 4 * g:4 * g + 4, :]))
                x0_dmas.append(nc.sync.dma_start(
                    out=x0_sb[:, 4 * g:4 * g + 4, :],
                    in_=xr0[:, 4 * g:4 * g + 4, :]))
            nc.gpsimd.dma_start(out=cos_sb[:], in_=cosd[:])
            nc.gpsimd.dma_start(out=sinm_sb[:], in_=sinmd[:])
            for p in range(n_patterns):
                nc.gpsimd.dma_start(out=cst[:, p * 128:(p + 1) * 128],
                                    in_=maskd[p])
            nc.vector.memset(ones8[:], 1.0)
            for g in range(4):
                di = nc.sync.dma_start(
                    out=wk_sb[:, 4 * g:4 * g + 4, :],
                    in_=wr["k"][:, 4 * g:4 * g + 4, :])
                for d0 in (x0_dmas[3], wq_dmas[3]):
                    add_dep_helper(di.ins, d0.ins, reason="defer wk")

            def rope(out_a, h, js):
                q = out_a[:, h, js]
                rq = tp.tile([128, QC], bf16, tag="rot")
                nc.sync.dma_start(out=rq[0:64, :], in_=out_a[64:128, h, js])
                nc.sync.dma_start(out=rq[64:128, :], in_=out_a[0:64, h, js])
                nc.vector.tensor_mul(rq[:], rq[:], sinm_sb[:, js])
                nc.vector.tensor_mul(q, q, cos_sb[:, js])
                nc.vector.tensor_add(q, q, rq[:])

            # QK pass: x chunk loaded once, Q then K, hc-outer, 8 banks
            for j in range(NQC):
                js = slice(j * QC, (j + 1) * QC)
                if j == 0:
                    x_sb = x0_sb
                else:
                    x_sb = xp.tile([128, NHC, QC], bf16, tag="x", name=f"x{j}")
                    xr = xT[:, js].rearrange("(a p) m -> p a m", p=128)
                    for g in range(4):
                        nc.sync.dma_start(out=x_sb[:, 4 * g:4 * g + 4, :],
                                          in_=xr[:, 4 * g:4 * g + 4, :])
                for w_sb, out_a, tag in ((wq_sb, QTa, "q"), (wk_sb, KTa, "k")):
                    for wv in range(2):
                        hs = range(4 * wv, 4 * wv + 4)
                        ps = {h: psA.tile([128, QC], fp32, tag="psA",
                                          name=f"ps{tag}{j}_{h}") for h in hs}
                        for hc in range(NHC):
                            for h in hs:
                                nc.tensor.matmul(
                                    ps[h][:],
                                    w_sb[:, hc, h * HD:(h + 1) * HD],
                                    x_sb[:, hc, :],
                                    start=(hc == 0), stop=(hc == NHC - 1))
                        for h in hs:
                            nc.scalar.copy(out_a[:, h, js], ps[h][:])
                            rope(out_a, h, js)

            # V pass: re-read x, pos-major, 8 banks, DVE copies
            wv_sb = wpool.tile([128, NHC, HPC * HD], bf16, tag="w", name="w_v")
            nc.sync.dma_start(out=wv_sb[:], in_=wr["v"][:])
            for j in range(NQC):
                js = slice(j * QC, (j + 1) * QC)
                x_sb = xp.tile([128, NHC, QC], bf16, tag="x", name=f"xv{j}")
                xr = xT[:, js].rearrange("(a p) m -> p a m", p=128)
                for g in range(4):
                    nc.sync.dma_start(out=x_sb[:, 4 * g:4 * g + 4, :],
                                      in_=xr[:, 4 * g:4 * g + 4, :])
                for wv in range(2):
                    pbs = (2 * wv, 2 * wv + 1)
                    psd = {(pb, dc): psA.tile([128, QC], fp32, tag="psA",
                                              name=f"psv{j}_{pb}_{dc}")
                           for pb in pbs for dc in range(2)}
                    for hc in range(NHC):
                        for pb in pbs:
                            for dc in range(2):
                                nc.tensor.matmul(
                                    psd[(pb, dc)][:],
                                    x_sb[:, hc, pb * 128:(pb + 1) * 128],
                                    wv_sb[:, hc, dc * QC:(dc + 1) * QC],
                                    start=(hc == 0), stop=(hc == NHC - 1))
                    for pb in pbs:
                        for dc in range(2):
                            nc.vector.tensor_copy(
                                Va[:, j * 4 + pb, dc * QC:(dc + 1) * QC],
                                psd[(pb, dc)][:])

            tp_cm.__exit__(None, None, None)
            xp_cm.__exit__(None, None, None)
            ropec_cm.__exit__(None, None, None)
            wpool_cm.__exit__(None, None, None)
            psA_cm.__exit__(None, None, None)

            # ---------------- phases B + C ----------------
            with tc.tile_pool(name="post", bufs=1, side="left") as post:
                OTa = post.tile([HD, HPC, L], bf16, tag="ota")
                wo_sb = post.tile([128, HPC, H], bf16, tag="wo")
                nc.sync.dma_start(
                    out=wo_sb[:],
                    in_=woT[:].rearrange("(a p) m -> p a m", p=128))

                _attention(tc, nc, plans, QTa, KTa, Va, OTa, cst, ones8,
                           fp32, bf16, fp8, EXP, DR)

                with tc.tile_pool(name="ysb", bufs=3, side="right") as ypool, \
                     tc.tile_pool(name="ps_c", bufs=4, space="PSUM") as ps_c:
                    for j in range(NQC):
                        for oc in range(H // 128):
                            ps = ps_c.tile([128, QC], fp32, tag="psc")
                            for fc in range(HPC):
                                nc.tensor.matmul(
                                    ps[:],
                                    wo_sb[:, fc, oc * 128:(oc + 1) * 128],
                                    OTa[:, fc, j * QC:(j + 1) * QC],
                                    start=(fc == 0), stop=(fc == HPC - 1))
                            y_sb = ypool.tile([128, QC], bf16, tag="y")
                            nc.vector.tensor_copy(y_sb[:], ps[:])
                            nc.sync.dma_start(
                                out=yT[oc * 128:(oc + 1) * 128,
                                       j * QC:(j + 1) * QC],
                                in_=y_sb[:])

    nc.compile()
    return nc


def _attention(tc, nc, plans, QTa, KTa, Va, OTa, cst, ones8,
               fp32, bf16, fp8, EXP, DR):
    """S ring -> grouped exp -> mask -> fp8 copy -> O + DR rowsums ->
    deferred per-head normalization."""
    with tc.tile_pool(name="pring", bufs=1, side="right") as prp, \
         tc.tile_pool(name="bcp", bufs=2, side="right") as bcp, \
         tc.tile_pool(name="rrp", bufs=4, side="right") as rrp, \
         tc.tile_pool(name="ps_s", bufs=1, space="PSUM") as ps_s, \
         tc.tile_pool(name="ps_o", bufs=1, space="PSUM") as ps_o, \
         tc.tile_pool(name="ps_r", bufs=1, space="PSUM") as ps_r:
        pss = ps_s.tile([128, 4 * QC], fp32, tag="pss")   # 4-bank S ring
        pso = [ps_o.tile([128, QC], fp32, tag=f"pso{i}", name=f"pso{i}")
               for i in range(2)]                         # O accum, 2 banks
        psr = [ps_r.tile([1, QC], fp32, tag=f"psr{i}", name=f"psr{i}")
               for i in range(2)]                         # rowsums, 2 banks
        P = prp.tile([128, 4, 2, QC], bf16, tag="P")      # 4 group-slots x 2
        ones_b = prp.tile([128, 1], bf16, tag="onesb")
        nc.vector.memset(ones_b[:], 1.0)
        Pf = P[:].rearrange("p g a n -> p (g a n)")

        def emit_group(it):
            h, j, blocks, grp, base, gs = (it["h"], it["j"], it["blocks"],
                                           it["grp"], it["base"], it["gs"])
            w0f = blocks[grp[0]][1]
            for gi, bi in enumerate(grp):
                i, w0, _ = blocks[bi]
                slot = base + gi
                lo = w0f if gi == 0 else 0
                nc.tensor.matmul(
                    pss[:, slot * QC + lo:(slot + 1) * QC],
                    KTa[:, h, i * 128:(i + 1) * 128],
                    QTa[:, h, j * QC + lo:j * QC + QC],
                    start=True, stop=True)
            span0 = base * QC + w0f
            span1 = (base + len(grp)) * QC
            po = gs * 2 * QC
            nc.scalar.activation(Pf[:, po + w0f:po + len(grp) * QC],
                                 pss[:, span0:span1], EXP, scale=SCALE)

        def consume_group(it):
            h, j, blocks, grp, gs = (it["h"], it["j"], it["blocks"],
                                     it["grp"], it["gs"])
            n = len(blocks)
            for gi, bi in enumerate(grp):
                i, w0, diags = blocks[bi]
                for t, pat in diags:
                    nc.vector.tensor_mul(
                        P[:, gs, gi, t * 128:(t + 1) * 128],
                        P[:, gs, gi, t * 128:(t + 1) * 128],
                        cst[:, pat * 128:(pat + 1) * 128])
            for gi, bi in enumerate(grp):
                i, w0, _ = blocks[bi]
                nc.tensor.matmul(
                    pso[j % 2][:, w0:QC],
                    Va[:, i, h * HD:(h + 1) * HD],
                    P[:, gs, gi, w0:QC],
                    start=(bi == 0), stop=(bi == n - 1))
            for gi, bi in enumerate(grp):
                i, w0, _ = blocks[bi]
                nc.tensor.matmul(
                    psr[j % 2][0:1, w0:QC],
                    ones_b[:, 0:1],
                    P[:, gs, gi, w0:QC],
                    start=(bi == 0), stop=(bi == n - 1))
            if grp[-1] == n - 1:
                # chunk epilogue: stash O, reciprocal of the denominator
                js = slice(j * QC, (j + 1) * QC)
                nc.vector.tensor_copy(OTa[:, h, js], pso[j % 2][:])
                r_sb = rrp.tile([1, QC], fp32, tag="r", name=f"r{h}_{j}")
                nc.vector.reciprocal_approx_fast(out=r_sb[0:1, :],
                                                 in_=psr[j % 2][0:1, :])
                rb_sb = rrp.tile([1, QC], bf16, tag="rb", name=f"rb{h}_{j}")
                nc.vector.tensor_copy(rb_sb[0:1, :], r_sb[0:1, :])
                nc.gpsimd.partition_broadcast(it["bc4"][:, j, :],
                                              rb_sb[0:1, :])
                if j == NQC - 1:
                    # head epilogue: batched normalization
                    nc.vector.tensor_mul(
                        OTa[:, h, :], OTa[:, h, :],
                        it["bc4"][:].rearrange("p a n -> p (a n)"))

        # flat group stream with two-group lookahead
        parity = 0
        gctr = 0
        pending = []
        for h in range(HPC):
            bc4 = bcp.tile([128, NQC, QC], bf16, tag="bc4", name=f"bc{h}")
            for j in range(NQC):
                blocks = plans[j]
                groups, parity2 = _plan_groups(blocks, parity)
                for grp in groups:
                    it = {"h": h, "j": j, "blocks": blocks, "grp": grp,
                          "base": (0, 2)[parity], "gs": gctr % 4, "bc4": bc4}
                    parity = 1 - parity
                    gctr += 1
                    emit_group(it)
                    pending.append(it)
                    if len(pending) > 2:
                        consume_group(pending.pop(0))
                assert parity == parity2
        for it in pending:
            consume_group(it)


def _prep_inputs(x, Wq, Wk, Wv, Wo, patterns):
    import ml_dtypes
    bf16 = ml_dtypes.bfloat16

    inv_freq = 1.0 / (ROPE_BASE ** (np.arange(0, HD, 2, dtype=np.float64)
                                    / HD))
    t = np.arange(L, dtype=np.float64)
    freqs = np.outer(t, inv_freq)
    emb = np.concatenate((freqs, freqs), axis=-1)
    cos = np.cos(emb).T.astype(np.float32)
    sin = np.sin(emb).T.astype(np.float32)
    sinm = sin.copy()
    sinm[0:64] = -sin[0:64]
    cos_b = cos.astype(bf16)
    sinm_b = sinm.astype(bf16)

    npat = max(len(patterns), 1)
    maskd = np.zeros((npat, 128, 128), dtype=bf16)
    for i, p in enumerate(patterns):
        maskd[i] = p.astype(np.float32).astype(bf16)

    in_maps = []
    for c in range(NCORES):
        b, half = c // 2, c % 2
        rows = slice(half * HPC * HD, (half + 1) * HPC * HD)
        in_maps.append({
            "xT": np.ascontiguousarray(x[b].T).astype(bf16),
            "wqT": np.ascontiguousarray(Wq[rows, :].T).astype(bf16),
            "wkT": np.ascontiguousarray(Wk[rows, :].T).astype(bf16),
            "wvT": np.ascontiguousarray(Wv[rows, :].T).astype(bf16),
            "woT": np.ascontiguousarray(Wo[:, rows].T).astype(bf16),
            "cosd": cos_b,
            "sinmd": sinm_b,
            "maskd": maskd,
        })
    return in_maps


def kernel(x, mask, Wq, Wk, Wv, Wo, _trace=False):
    from concourse.bass_utils import run_bass_kernel_spmd

    x = np.asarray(x, dtype=np.float32)
    mask2d = np.asarray(mask, dtype=np.int32).reshape(L, L)
    key = mask2d.tobytes()
    if key not in _cache:
        kind, patterns, block_pat = _analyze_mask(mask2d)
        nc = _build(kind, block_pat, len(patterns))
        _cache[key] = (nc, patterns)
    nc, patterns = _cache[key]

    in_maps = _prep_inputs(x, np.asarray(Wq, np.float32),
                           np.asarray(Wk, np.float32),
                           np.asarray(Wv, np.float32),
                           np.asarray(Wo, np.float32), patterns)
    res = run_bass_kernel_spmd(nc, in_maps, list(range(NCORES)),
                               trace=_trace)
    y = np.empty((B, L, H), dtype=np.float32)
    for b in range(B):
        acc = res.results[2 * b]["yT"].astype(np.float32) + \
              res.results[2 * b + 1]["yT"].astype(np.float32)
        y[b] = acc.T
    if _trace:
        kernel.last_results = res
    return y


if __name__ == "__main__":
    import reference
    inputs = reference.setup_inputs()
    inputs = {k: np.asarray(v) for k, v in inputs.items()}
    out = kernel(**inputs)
    exp = np.asarray(reference.reference(**{k: v for k, v in inputs.items()}))
    err = np.abs(out - exp).max() / np.abs(exp).max()
    print("rel err (absmax):", err)


# revision 11
# speedup vs baseline: 1.0989x; 1.0682x over previous
"""Trainium2 Bass kernel: causal multi-head attention with RoPE.

Model: B=4, L=2048, H=2048, NH=16 heads, head_dim=128.
  q = x @ Wq.T ; k = x @ Wk.T ; v = x @ Wv.T        (per-head split)
  q, k <- RoPE(q, k)
  attn = softmax(mask(q k^T / sqrt(hd)))
  out  = (attn @ v) heads-concat @ Wo.T

Sharding (8 cores): hybrid batch x tensor-parallel.  Core c handles
batch b = c//2 and heads half*8..half*8+7 with half = c%2.  Wq/Wk/Wv are
column-sharded (8 heads per core), Wo row-sharded; each core produces a
partial y[b] (bf16) and the host sums the two partials per batch.

Per-core dataflow (SBUF-resident, bf16 inputs / fp32 accumulation):
  phase A: QK pass per 512-pos x chunk (x loaded once for both):
           hc-outer accumulation into 8 PSUM banks so the first matmul
           needs only 1.5MB of DMA; fused RoPE (partition-swap DMA +
           DVE).  Then a V pass (pos-major, 8 banks) with PSUM->SBUF
           copies on DVE so ACT is drained before attention.
  phase B: flash-style causal attention per (head, 512-wide q chunk):
           S^T tiles into a 5-bank PSUM ring, exp on ACT over multi-bank
           group spans (group sizes DP-chosen to trade ACT instruction
           overhead vs dead-margin columns), triangular-mask multiply on
           diagonal blocks (DVE), O^T accumulation (PE), softmax
           denominator via fp8 DoubleRow ones-matmul over pair-packed
           fp8 copies of P (DVE converts), per-head batched
           normalization (unnormalized O commutes with the denominator).
  phase C: y^T partial = Wo_shard O^T (PE) -> DRAM bf16.
"""

import math
import numpy as np

B, L, H, NH, HD = 4, 2048, 2048, 16, 128
ROPE_BASE = 10000.0
NCORES = 8
HPC = 8          # heads per core
QC = 512         # q chunk width
NQC = L // QC    # 4 q chunks
NKB = L // 128   # 16 kp blocks
SCALE = 1.0 / math.sqrt(HD)
NHC = H // 128   # 16 input-feature blocks

_cache = {}


def _analyze_mask(mask2d):
    """Classify each (q_block, kp_block) 128x128 block of the [L, L] mask."""
    nb = L // 128
    kind = [[0] * nb for _ in range(nb)]
    patterns = []
    pat_key_to_idx = {}
    block_pat = {}
    for qb in range(nb):
        rows = mask2d[qb * 128:(qb + 1) * 128]
        for kb in range(nb):
            blk = rows[:, kb * 128:(kb + 1) * 128]
            s = int(blk.sum())
            if s == 0:
                kind[qb][kb] = 0
            elif s == 128 * 128:
                kind[qb][kb] = 1
            else:
                kind[qb][kb] = 2
                key = blk.tobytes()
                idx = pat_key_to_idx.get(key)
                if idx is None:
                    idx = len(patterns)
                    pat_key_to_idx[key] = idx
                    # stored transposed: S^T tiles are [kp, q]
                    patterns.append(np.ascontiguousarray(blk.T))
                block_pat[(qb, kb)] = idx
    return kind, patterns, block_pat


def _chunk_plan(kind, block_pat):
    """Per q-chunk block list: (i, w0, [(t, pat), ...]) for live kp blocks.

    w0 = first live 128-col offset within the chunk; requires the causal
    staircase (w0 nondecreasing in i, first block full, even count)."""
    plans = []
    for j in range(NQC):
        blocks = []
        for i in range(NKB):
            live = [t for t in range(4) if kind[4 * j + t][i] != 0]
            if not live:
                continue
            w0 = live[0] * 128
            assert live == list(range(live[0], 4)), "non-staircase mask"
            diags = [(t, block_pat[(4 * j + t, i)]) for t in live
                     if kind[4 * j + t][i] == 2]
            blocks.append((i, w0, diags))
        assert blocks and blocks[0][1] == 0, "first live block must be full"
        assert len(blocks) % 2 == 0, "need even live-block count per chunk"
        for a, b in zip(blocks, blocks[1:]):
            assert a[1] <= b[1], "w0 must be nondecreasing"
        plans.append(blocks)
    return plans


def _plan_groups(blocks, parity0):
    """Split a chunk's blocks into exp groups for the 5-slot PSUM ring.

    Ring = A slots (0,1) + B slots (2,3,4), strictly alternating; a group
    may underfill.  DP minimizes ACT cost = sum(OH + len*512 - w0_first).
    Returns (groups, parity_out), groups = lists of indices into blocks."""
    OH = 390
    n = len(blocks)
    INF = float("inf")
    dp = [[None, None] for _ in range(n + 1)]
    dp[n] = [(0, 0), (0, 0)]
    for i in range(n - 1, -1, -1):
        for p in (0, 1):
            cap = 2
            best = (INF, 0)
            for ln in range(1, min(cap, n - i) + 1):
                cost = OH + ln * 512 - blocks[i][1] + dp[i + ln][1 - p][0]
                if cost < best[0]:
                    best = (cost, ln)
            dp[i][p] = best
    groups = []
    i, p = 0, parity0
    while i < n:
        ln = dp[i][p][1]
        groups.append(list(range(i, i + ln)))
        i += ln
        p = 1 - p
    return groups, p


def _build(kind, block_pat, n_patterns):
    """Build the SPMD bass program (same for all 8 cores)."""
    import concourse.bacc as bacc
    import concourse.mybir as mybir
    import concourse.tile as tile
    from concourse.tile import add_dep_helper

    fp32 = mybir.dt.float32
    bf16 = mybir.dt.bfloat16
    fp8 = mybir.dt.float8e4
    EXP = mybir.ActivationFunctionType.Exp
    DR = mybir.MatmulPerfMode.DoubleRow

    nc = bacc.Bacc("TRN2", target_bir_lowering=False, debug=False)

    xT = nc.dram_tensor("xT", [H, L], bf16, kind="ExternalInput")
    wqT = nc.dram_tensor("wqT", [H, HPC * HD], bf16, kind="ExternalInput")
    wkT = nc.dram_tensor("wkT", [H, HPC * HD], bf16, kind="ExternalInput")
    wvT = nc.dram_tensor("wvT", [H, HPC * HD], bf16, kind="ExternalInput")
    woT = nc.dram_tensor("woT", [HPC * HD, H], bf16, kind="ExternalInput")
    cosd = nc.dram_tensor("cosd", [HD, L], bf16, kind="ExternalInput")
    sinmd = nc.dram_tensor("sinmd", [HD, L], bf16, kind="ExternalInput")
    npat = max(n_patterns, 1)
    maskd = nc.dram_tensor("maskd", [npat, 128, 128], bf16, kind="ExternalInput")
    yT = nc.dram_tensor("yT", [H, L], bf16, kind="ExternalOutput")

    plans = _chunk_plan(kind, block_pat)

    with tile.TileContext(nc) as tc:
        with tc.tile_pool(name="persist", bufs=1, side="left") as persist:
            cst = persist.tile([128, npat * 128], bf16, tag="cst")
            ones8 = persist.tile([128, 2, 16], fp8, tag="ones8")
            QTa = persist.tile([HD, HPC, L], bf16, tag="qta")
            KTa = persist.tile([HD, HPC, L], bf16, tag="kta")
            Va = persist.tile([128, NKB, HPC * HD], bf16, tag="va")

            # ---------------- phase A ----------------
            wpool_cm = tc.tile_pool(name="wpool", bufs=2, side="right")
            wpool = wpool_cm.__enter__()
            ropec_cm = tc.tile_pool(name="ropec", bufs=1, side="right")
            ropec = ropec_cm.__enter__()
            xp_cm = tc.tile_pool(name="xp", bufs=2, side="right")
            xp = xp_cm.__enter__()
            tp_cm = tc.tile_pool(name="tpool", bufs=2, side="right")
            tp = tp_cm.__enter__()
            psA_cm = tc.tile_pool(name="psA", bufs=8, space="PSUM")
            psA = psA_cm.__enter__()

            wq_sb = wpool.tile([128, NHC, HPC * HD], bf16, tag="w", name="w_q")
            wk_sb = wpool.tile([128, NHC, HPC * HD], bf16, tag="w", name="w_k")
            cos_sb = ropec.tile([HD, L], bf16, tag="cos")
            sinm_sb = ropec.tile([HD, L], bf16, tag="sinm")

            wr = {"q": wqT[:].rearrange("(a p) m -> p a m", p=128),
                  "k": wkT[:].rearrange("(a p) m -> p a m", p=128),
                  "v": wvT[:].rearrange("(a p) m -> p a m", p=128)}

            # startup: interleave x(j0) and wq groups; defer the rest
            x0_sb = xp.tile([128, NHC, QC], bf16, tag="x", name="x0")
            xr0 = xT[:, 0:QC].rearrange("(a p) m -> p a m", p=128)
            x0_dmas, wq_dmas = [], []
            for g in range(4):
                wq_dmas.append(nc.sync.dma_start(
                    out=wq_sb[:, 4 * g:4 * g + 4, :],
                    in_=wr["q"][:, 4 * g:4 * g + 4, :]))
                x0_dmas.append(nc.sync.dma_start(
                    out=x0_sb[:, 4 * g:4 * g + 4, :],
                    in_=xr0[:, 4 * g:4 * g + 4, :]))
            nc.gpsimd.dma_start(out=cos_sb[:], in_=cosd[:])
            nc.gpsimd.dma_start(out=sinm_sb[:], in_=sinmd[:])
            for p in range(n_patterns):
                nc.gpsimd.dma_start(out=cst[:, p * 128:(p + 1) * 128],
                                    in_=maskd[p])
            nc.vector.memset(ones8[:], 1.0)
            for g in range(4):
                di = nc.sync.dma_start(
                    out=wk_sb[:, 4 * g:4 * g + 4, :],
                    in_=wr["k"][:, 4 * g:4 * g + 4, :])
                for d0 in (x0_dmas[3], wq_dmas[3]):
                    add_dep_helper(di.ins, d0.ins, reason="defer wk")

            def rope(out_a, h, js):
                q = out_a[:, h, js]
                rq = tp.tile([128, QC], bf16, tag="rot")
                nc.sync.dma_start(out=rq[0:64, :], in_=out_a[64:128, h, js])
                nc.sync.dma_start(out=rq[64:128, :], in_=out_a[0:64, h, js])
                nc.vector.tensor_mul(rq[:], rq[:], sinm_sb[:, js])
                nc.vector.tensor_mul(q, q, cos_sb[:, js])
                nc.vector.tensor_add(q, q, rq[:])

            # QK pass: x chunk loaded once, Q then K, hc-outer, 8 banks
            for j in range(NQC):
                js = slice(j * QC, (j + 1) * QC)
                if j == 0:
                    x_sb = x0_sb
                else:
                    x_sb = xp.tile([128, NHC, QC], bf16, tag="x", name=f"x{j}")
                    xr = xT[:, js].rearrange("(a p) m -> p a m", p=128)
                    for g in range(4):
                        nc.sync.dma_start(out=x_sb[:, 4 * g:4 * g + 4, :],
                                          in_=xr[:, 4 * g:4 * g + 4, :])
                for w_sb, out_a, tag in ((wq_sb, QTa, "q"), (wk_sb, KTa, "k")):
                    for wv in range(2):
                        hs = range(4 * wv, 4 * wv + 4)
                        ps = {h: psA.tile([128, QC], fp32, tag="psA",
                                          name=f"ps{tag}{j}_{h}") for h in hs}
                        for hc in range(NHC):
                            for h in hs:
                                nc.tensor.matmul(
                                    ps[h][:],
                                    w_sb[:, hc, h * HD:(h + 1) * HD],
                                    x_sb[:, hc, :],
                                    start=(hc == 0), stop=(hc == NHC - 1))
                        for h in hs:
                            nc.scalar.copy(out_a[:, h, js], ps[h][:])
                            rope(out_a, h, js)

            # V pass: re-read x, pos-major, 8 banks, DVE copies
            wv_sb = wpool.tile([128, NHC, HPC * HD], bf16, tag="w", name="w_v")
            nc.sync.dma_start(out=wv_sb[:], in_=wr["v"][:])
            for j in range(NQC):
                js = slice(j * QC, (j + 1) * QC)
                x_sb = xp.tile([128, NHC, QC], bf16, tag="x", name=f"xv{j}")
                xr = xT[:, js].rearrange("(a p) m -> p a m", p=128)
                for g in range(4):
                    nc.sync.dma_start(out=x_sb[:, 4 * g:4 * g + 4, :],
                                      in_=xr[:, 4 * g:4 * g + 4, :])
                for wv in range(2):
                    pbs = (2 * wv, 2 * wv + 1)
                    psd = {(pb, dc): psA.tile([128, QC], fp32, tag="psA",
                                              name=f"psv{j}_{pb}_{dc}")
                           for pb in pbs for dc in range(2)}
                    for hc in range(NHC):
                        for pb in pbs:
                            for dc in range(2):
                                nc.tensor.matmul(
                                    psd[(pb, dc)][:],
                                    x_sb[:, hc, pb * 128:(pb + 1) * 128],
                                    wv_sb[:, hc, dc * QC:(dc + 1) * QC],
                                    start=(hc == 0), stop=(hc == NHC - 1))
                    for pb in pbs:
                        for dc in range(2):
                            nc.vector.tensor_copy(
                                Va[:, j * 4 + pb, dc * QC:(dc + 1) * QC],
                                psd[(pb, dc)][:])

            tp_cm.__exit__(None, None, None)
            xp_cm.__exit__(None, None, None)
            ropec_cm.__exit__(None, None, None)
            wpool_cm.__exit__(None, None, None)
            psA_cm.__exit__(None, None, None)

            # ---------------- phases B + C ----------------
            with tc.tile_pool(name="post", bufs=1, side="left") as post:
                OTa = post.tile([HD, HPC, L], bf16, tag="ota")
                wo_sb = post.tile([128, HPC, H], bf16, tag="wo")
                nc.sync.dma_start(
                    out=wo_sb[:],
                    in_=woT[:].rearrange("(a p) m -> p a m", p=128))

                _attention(tc, nc, kind, block_pat, QTa, KTa, Va, OTa,
                           cst, fp32, bf16, EXP)

                with tc.tile_pool(name="ysb", bufs=3, side="right") as ypool, \
                     tc.tile_pool(name="ps_c", bufs=4, space="PSUM") as ps_c:
                    for j in range(NQC):
                        for oc in range(H // 128):
                            ps = ps_c.tile([128, QC], fp32, tag="psc")
                            for fc in range(HPC):
                                nc.tensor.matmul(
                                    ps[:],
                                    wo_sb[:, fc, oc * 128:(oc + 1) * 128],
                                    OTa[:, fc, j * QC:(j + 1) * QC],
                                    start=(fc == 0), stop=(fc == HPC - 1))
                            y_sb = ypool.tile([128, QC], bf16, tag="y")
                            nc.vector.tensor_copy(y_sb[:], ps[:])
                            nc.sync.dma_start(
                                out=yT[oc * 128:(oc + 1) * 128,
                                       j * QC:(j + 1) * QC],
                                in_=y_sb[:])

    nc.compile()
    return nc


def _attention(tc, nc, kind, block_pat, QTa, KTa, Va, OTa, cst,
               fp32, bf16, EXP):
    """v1-style jpair attention: q-chunk PAIRS inside the kp-block loop so
    S (and O, rowsum) matmuls sit back-to-back with a shared stationary
    operand; one kp-block of lookahead keeps PE ahead of the ACT exp."""
    with tc.tile_pool(name="pp", bufs=6, side="right") as ppool, \
         tc.tile_pool(name="rr", bufs=4, side="right") as rpool, \
         tc.tile_pool(name="bb", bufs=4, side="right") as bpool, \
         tc.tile_pool(name="on", bufs=1, side="right") as onp, \
         tc.tile_pool(name="ps_s", bufs=4, space="PSUM") as ps_s, \
         tc.tile_pool(name="ps_o", bufs=1, space="PSUM") as ps_o, \
         tc.tile_pool(name="ps_r", bufs=1, space="PSUM") as ps_r:
        ones_sb = onp.tile([128, 1], bf16, tag="onesb")
        nc.vector.memset(ones_sb[:], 1.0)
        for h in range(HPC):
            for jpair in ((0, 1), (2, 3)):
                blocks_j = {}
                first_i = {}
                last_i = {}
                for j in jpair:
                    for i in range(NKB):
                        live = [t for t in range(4)
                                if kind[4 * j + t][i] != 0]
                        if live:
                            blocks_j.setdefault(i, []).append((j, live))
                            if j not in first_i:
                                first_i[j] = i
                            last_i[j] = i
                if not first_i:
                    continue
                pso = {j: ps_o.tile([128, QC], fp32, tag=f"pso{j % 2}",
                                    name=f"pso{h}_{j}")
                       for j in first_i}
                psr = {j: ps_r.tile([1, QC], fp32, tag=f"psr{j % 2}",
                                    name=f"psr{h}_{j}")
                       for j in first_i}

                def emit_s(i, j, live):
                    t0, t1 = live[0], live[-1]
                    w0, w1 = t0 * 128, (t1 + 1) * 128
                    pss = ps_s.tile([128, QC], fp32, tag="pss",
                                    name=f"pss{h}_{j}_{i}")
                    nc.tensor.matmul(
                        pss[:, w0:w1],
                        KTa[:, h, i * 128:(i + 1) * 128],
                        QTa[:, h, j * QC + w0:j * QC + w1],
                        start=True, stop=True)
                    P = ppool.tile([128, QC], bf16, tag="p",
                                   name=f"p{h}_{j}_{i}")
                    first = (first_i[j] == i)
                    if w0 > 0 and first:
                        nc.vector.memset(P[:, 0:w0], 0.0)
                    if w1 < QC and first:
                        nc.vector.memset(P[:, w1:QC], 0.0)
                    nc.scalar.activation(P[:, w0:w1], pss[:, w0:w1],
                                         EXP, scale=SCALE)
                    for t in range(t0, t1 + 1):
                        qb = 4 * j + t
                        if kind[qb][i] == 0:
                            nc.vector.memset(
                                P[:, t * 128:(t + 1) * 128], 0.0)
                        elif kind[qb][i] == 2:
                            pat = block_pat[(qb, i)]
                            nc.vector.tensor_mul(
                                P[:, t * 128:(t + 1) * 128],
                                P[:, t * 128:(t + 1) * 128],
                                cst[:, pat * 128:(pat + 1) * 128])
                    return (j, P, w0, first)

                def emit_ovr(i, group):
                    for j, P, w0, first in group:
                        m0 = 0 if first else w0
                        nc.tensor.matmul(
                            pso[j][:, m0:QC],
                            Va[:, i, h * HD:(h + 1) * HD],
                            P[:, m0:QC],
                            start=first, stop=(last_i[j] == i))
                    for j, P, w0, first in group:
                        m0 = 0 if first else w0
                        nc.tensor.matmul(
                            psr[j][0:1, m0:QC], ones_sb[:, 0:1], P[:, m0:QC],
                            start=first, stop=(last_i[j] == i))
                    for j, P, w0, first in group:
                        if last_i[j] != i:
                            continue
                        r_sb = rpool.tile([128, QC], fp32, tag="r",
                                          name=f"r{h}_{j}")
                        nc.vector.reciprocal_approx_fast(
                            out=r_sb[0:1, :], in_=psr[j][0:1, :])
                        rb_sb = rpool.tile([128, QC], bf16, tag="rb",
                                           name=f"rb{h}_{j}")
                        nc.vector.tensor_copy(rb_sb[0:1, :],
                                              r_sb[0:1, :])
                        bc_sb = bpool.tile([128, QC], bf16, tag="bc",
                                           name=f"bc{h}_{j}")
                        nc.gpsimd.partition_broadcast(bc_sb[:],
                                                      rb_sb[0:1, :])
                        nc.vector.tensor_mul(
                            OTa[:, h, j * QC:(j + 1) * QC],
                            pso[j][:], bc_sb[:])

                prev = None
                for i in sorted(blocks_j):
                    cur = (i, [emit_s(i, j, live)
                               for j, live in blocks_j[i]])
                    if prev is not None:
                        emit_ovr(*prev)
                    prev = cur
                if prev is not None:
                    emit_ovr(*prev)


def _prep_inputs(x, Wq, Wk, Wv, Wo, patterns):
    import ml_dtypes
    bf16 = ml_dtypes.bfloat16

    inv_freq = 1.0 / (ROPE_BASE ** (np.arange(0, HD, 2, dtype=np.float64)
                                    / HD))
    t = np.arange(L, dtype=np.float64)
    freqs = np.outer(t, inv_freq)
    emb = np.concatenate((freqs, freqs), axis=-1)
    cos = np.cos(emb).T.astype(np.float32)
    sin = np.sin(emb).T.astype(np.float32)
    sinm = sin.copy()
    sinm[0:64] = -sin[0:64]
    cos_b = cos.astype(bf16)
    sinm_b = sinm.astype(bf16)

    npat = max(len(patterns), 1)
    maskd = np.zeros((npat, 128, 128), dtype=bf16)
    for i, p in enumerate(patterns):
        maskd[i] = p.astype(np.float32).astype(bf16)

    in_maps = []
    for c in range(NCORES):
        b, half = c // 2, c % 2
        rows = slice(half * HPC * HD, (half + 1) * HPC * HD)
        in_maps.append({
            "xT": np.ascontiguousarray(x[b].T).astype(bf16),
            "wqT": np.ascontiguousarray(Wq[rows, :].T).astype(bf16),
            "wkT": np.ascontiguousarray(Wk[rows, :].T).astype(bf16),
            "wvT": np.ascontiguousarray(Wv[rows, :].T).astype(bf16),
            "woT": np.ascontiguousarray(Wo[:, rows].T).astype(bf16),
            "cosd": cos_b,
            "sinmd": sinm_b,
            "maskd": maskd,
        })
    return in_maps


def kernel(x, mask, Wq, Wk, Wv, Wo, _trace=False):
    from concourse.bass_utils import run_bass_kernel_spmd

    x = np.asarray(x, dtype=np.float32)
    mask2d = np.asarray(mask, dtype=np.int32).reshape(L, L)
    key = mask2d.tobytes()
    if key not in _cache:
        kind, patterns, block_pat = _analyze_mask(mask2d)
        nc = _build(kind, block_pat, len(patterns))
        _cache[key] = (nc, patterns)
    nc, patterns = _cache[key]

    in_maps = _prep_inputs(x, np.asarray(Wq, np.float32),
                           np.asarray(Wk, np.float32),
                           np.asarray(Wv, np.float32),
                           np.asarray(Wo, np.float32), patterns)
    res = run_bass_kernel_spmd(nc, in_maps, list(range(NCORES)),
                               trace=_trace)
    y = np.empty((B, L, H), dtype=np.float32)
    for b in range(B):
        acc = res.results[2 * b]["yT"].astype(np.float32) + \
              res.results[2 * b + 1]["yT"].astype(np.float32)
        y[b] = acc.T
    if _trace:
        kernel.last_results = res
    return y


if __name__ == "__main__":
    import reference
    inputs = reference.setup_inputs()
    inputs = {k: np.asarray(v) for k, v in inputs.items()}
    out = kernel(**inputs)
    exp = np.asarray(reference.reference(**{k: v for k, v in inputs.items()}))
    err = np.abs(out - exp).max() / np.abs(exp).max()
    print("rel err (absmax):", err)


# revision 12
# speedup vs baseline: 1.1280x; 1.0266x over previous
"""Trainium2 Bass kernel: causal multi-head attention with RoPE.

Model: B=4, L=2048, H=2048, NH=16 heads, head_dim=128.
  q = x @ Wq.T ; k = x @ Wk.T ; v = x @ Wv.T        (per-head split)
  q, k <- RoPE(q, k)
  attn = softmax(mask(q k^T / sqrt(hd)))
  out  = (attn @ v) heads-concat @ Wo.T

Sharding (8 cores): hybrid batch x tensor-parallel.  Core c handles
batch b = c//2 and heads half*8..half*8+7 with half = c%2.  Wq/Wk/Wv are
column-sharded (8 heads per core), Wo row-sharded; each core produces a
partial y[b] (bf16) and the host sums the two partials per batch.

Per-core dataflow (SBUF-resident, bf16 inputs / fp32 accumulation):
  phase A: QK pass per 512-pos x chunk (x loaded once for both):
           hc-outer accumulation into 8 PSUM banks so the first matmul
           needs only 1.5MB of DMA; fused RoPE (partition-swap DMA +
           DVE).  Then a V pass (pos-major, 8 banks) with PSUM->SBUF
           copies on DVE so ACT is drained before attention.
  phase B: flash-style causal attention per (head, 512-wide q chunk):
           S^T tiles into a 5-bank PSUM ring, exp on ACT over multi-bank
           group spans (group sizes DP-chosen to trade ACT instruction
           overhead vs dead-margin columns), triangular-mask multiply on
           diagonal blocks (DVE), O^T accumulation (PE), softmax
           denominator via fp8 DoubleRow ones-matmul over pair-packed
           fp8 copies of P (DVE converts), per-head batched
           normalization (unnormalized O commutes with the denominator).
  phase C: y^T partial = Wo_shard O^T (PE) -> DRAM bf16.
"""

import math
import numpy as np

B, L, H, NH, HD = 4, 2048, 2048, 16, 128
ROPE_BASE = 10000.0
NCORES = 8
HPC = 8          # heads per core
QC = 512         # q chunk width
NQC = L // QC    # 4 q chunks
NKB = L // 128   # 16 kp blocks
SCALE = 1.0 / math.sqrt(HD)
NHC = H // 128   # 16 input-feature blocks

_cache = {}


def _analyze_mask(mask2d):
    """Classify each (q_block, kp_block) 128x128 block of the [L, L] mask."""
    nb = L // 128
    kind = [[0] * nb for _ in range(nb)]
    patterns = []
    pat_key_to_idx = {}
    block_pat = {}
    for qb in range(nb):
        rows = mask2d[qb * 128:(qb + 1) * 128]
        for kb in range(nb):
            blk = rows[:, kb * 128:(kb + 1) * 128]
            s = int(blk.sum())
            if s == 0:
                kind[qb][kb] = 0
            elif s == 128 * 128:
                kind[qb][kb] = 1
            else:
                kind[qb][kb] = 2
                key = blk.tobytes()
                idx = pat_key_to_idx.get(key)
                if idx is None:
                    idx = len(patterns)
                    pat_key_to_idx[key] = idx
                    # stored transposed: S^T tiles are [kp, q]
                    patterns.append(np.ascontiguousarray(blk.T))
                block_pat[(qb, kb)] = idx
    return kind, patterns, block_pat


def _chunk_plan(kind, block_pat):
    """Per q-chunk block list: (i, w0, [(t, pat), ...]) for live kp blocks.

    w0 = first live 128-col offset within the chunk; requires the causal
    staircase (w0 nondecreasing in i, first block full, even count)."""
    plans = []
    for j in range(NQC):
        blocks = []
        for i in range(NKB):
            live = [t for t in range(4) if kind[4 * j + t][i] != 0]
            if not live:
                continue
            w0 = live[0] * 128
            assert live == list(range(live[0], 4)), "non-staircase mask"
            diags = [(t, block_pat[(4 * j + t, i)]) for t in live
                     if kind[4 * j + t][i] == 2]
            blocks.append((i, w0, diags))
        assert blocks and blocks[0][1] == 0, "first live block must be full"
        assert len(blocks) % 2 == 0, "need even live-block count per chunk"
        for a, b in zip(blocks, blocks[1:]):
            assert a[1] <= b[1], "w0 must be nondecreasing"
        plans.append(blocks)
    return plans


def _plan_groups(blocks, parity0):
    """Split a chunk's blocks into exp groups for the 5-slot PSUM ring.

    Ring = A slots (0,1) + B slots (2,3,4), strictly alternating; a group
    may underfill.  DP minimizes ACT cost = sum(OH + len*512 - w0_first).
    Returns (groups, parity_out), groups = lists of indices into blocks."""
    OH = 390
    n = len(blocks)
    INF = float("inf")
    dp = [[None, None] for _ in range(n + 1)]
    dp[n] = [(0, 0), (0, 0)]
    for i in range(n - 1, -1, -1):
        for p in (0, 1):
            cap = 2
            best = (INF, 0)
            for ln in range(1, min(cap, n - i) + 1):
                cost = OH + ln * 512 - blocks[i][1] + dp[i + ln][1 - p][0]
                if cost < best[0]:
                    best = (cost, ln)
            dp[i][p] = best
    groups = []
    i, p = 0, parity0
    while i < n:
        ln = dp[i][p][1]
        groups.append(list(range(i, i + ln)))
        i += ln
        p = 1 - p
    return groups, p


def _build(kind, block_pat, n_patterns):
    """Build the SPMD bass program (same for all 8 cores)."""
    import concourse.bacc as bacc
    import concourse.mybir as mybir
    import concourse.tile as tile
    from concourse.tile import add_dep_helper

    fp32 = mybir.dt.float32
    bf16 = mybir.dt.bfloat16
    fp8 = mybir.dt.float8e4
    EXP = mybir.ActivationFunctionType.Exp
    DR = mybir.MatmulPerfMode.DoubleRow

    nc = bacc.Bacc("TRN2", target_bir_lowering=False, debug=False)

    xT = nc.dram_tensor("xT", [H, L], bf16, kind="ExternalInput")
    wqT = nc.dram_tensor("wqT", [H, HPC * HD], bf16, kind="ExternalInput")
    wkT = nc.dram_tensor("wkT", [H, HPC * HD], bf16, kind="ExternalInput")
    wvT = nc.dram_tensor("wvT", [H, HPC * HD], bf16, kind="ExternalInput")
    woT = nc.dram_tensor("woT", [HPC * HD, H], bf16, kind="ExternalInput")
    cosd = nc.dram_tensor("cosd", [HD, L], bf16, kind="ExternalInput")
    sinmd = nc.dram_tensor("sinmd", [HD, L], bf16, kind="ExternalInput")
    npat = max(n_patterns, 1)
    maskd = nc.dram_tensor("maskd", [npat, 128, 128], bf16, kind="ExternalInput")
    yT = nc.dram_tensor("yT", [H, L], bf16, kind="ExternalOutput")

    plans = _chunk_plan(kind, block_pat)

    with tile.TileContext(nc) as tc:
        with tc.tile_pool(name="persist", bufs=1, side="left") as persist:
            cst = persist.tile([128, npat * 128], bf16, tag="cst")
            ones8 = persist.tile([128, 2, 16], fp8, tag="ones8")
            QTa = persist.tile([HD, HPC, L], bf16, tag="qta")
            KTa = persist.tile([HD, HPC, L], bf16, tag="kta")
            Va = persist.tile([128, NKB, HPC * HD], bf16, tag="va")

            # ---------------- phase A ----------------
            wpool_cm = tc.tile_pool(name="wpool", bufs=2, side="right")
            wpool = wpool_cm.__enter__()
            ropec_cm = tc.tile_pool(name="ropec", bufs=1, side="right")
            ropec = ropec_cm.__enter__()
            xp_cm = tc.tile_pool(name="xp", bufs=2, side="right")
            xp = xp_cm.__enter__()
            tp_cm = tc.tile_pool(name="tpool", bufs=2, side="right")
            tp = tp_cm.__enter__()
            psA_cm = tc.tile_pool(name="psA", bufs=8, space="PSUM")
            psA = psA_cm.__enter__()

            wq_sb = wpool.tile([128, NHC, HPC * HD], bf16, tag="w", name="w_q")
            wk_sb = wpool.tile([128, NHC, HPC * HD], bf16, tag="w", name="w_k")
            cos_sb = ropec.tile([HD, L], bf16, tag="cos")
            sinm_sb = ropec.tile([HD, L], bf16, tag="sinm")

            wr = {"q": wqT[:].rearrange("(a p) m -> p a m", p=128),
                  "k": wkT[:].rearrange("(a p) m -> p a m", p=128),
                  "v": wvT[:].rearrange("(a p) m -> p a m", p=128)}

            # startup: interleave x(j0) and wq groups; defer the rest
            x0_sb = xp.tile([128, NHC, QC], bf16, tag="x", name="x0")
            xr0 = xT[:, 0:QC].rearrange("(a p) m -> p a m", p=128)
            x0_dmas, wq_dmas = [], []
            for g in range(4):
                wq_dmas.append(nc.sync.dma_start(
                    out=wq_sb[:, 4 * g:4 * g + 4, :],
                    in_=wr["q"][:, 4 * g:4 * g + 4, :]))
                x0_dmas.append(nc.sync.dma_start(
                    out=x0_sb[:, 4 * g:4 * g + 4, :],
                    in_=xr0[:, 4 * g:4 * g + 4, :]))
            nc.gpsimd.dma_start(out=cos_sb[:], in_=cosd[:])
            nc.gpsimd.dma_start(out=sinm_sb[:], in_=sinmd[:])
            for p in range(n_patterns):
                nc.gpsimd.dma_start(out=cst[:, p * 128:(p + 1) * 128],
                                    in_=maskd[p])
            nc.vector.memset(ones8[:], 1.0)
            for g in range(4):
                di = nc.sync.dma_start(
                    out=wk_sb[:, 4 * g:4 * g + 4, :],
                    in_=wr["k"][:, 4 * g:4 * g + 4, :])
                for d0 in (x0_dmas[3], wq_dmas[3]):
                    add_dep_helper(di.ins, d0.ins, reason="defer wk")

            def rope(out_a, h, js):
                q = out_a[:, h, js]
                rq = tp.tile([128, QC], bf16, tag="rot")
                nc.sync.dma_start(out=rq[0:64, :], in_=out_a[64:128, h, js])
                nc.sync.dma_start(out=rq[64:128, :], in_=out_a[0:64, h, js])
                nc.vector.tensor_mul(rq[:], rq[:], sinm_sb[:, js])
                nc.vector.tensor_mul(q, q, cos_sb[:, js])
                nc.vector.tensor_add(q, q, rq[:])

            # QK pass: x chunk loaded once, Q then K, hc-outer, 8 banks
            for j in range(NQC):
                js = slice(j * QC, (j + 1) * QC)
                if j == 0:
                    x_sb = x0_sb
                else:
                    x_sb = xp.tile([128, NHC, QC], bf16, tag="x", name=f"x{j}")
                    xr = xT[:, js].rearrange("(a p) m -> p a m", p=128)
                    for g in range(4):
                        nc.sync.dma_start(out=x_sb[:, 4 * g:4 * g + 4, :],
                                          in_=xr[:, 4 * g:4 * g + 4, :])
                for w_sb, out_a, tag in ((wq_sb, QTa, "q"), (wk_sb, KTa, "k")):
                    for wv in range(2):
                        hs = range(4 * wv, 4 * wv + 4)
                        ps = {h: psA.tile([128, QC], fp32, tag="psA",
                                          name=f"ps{tag}{j}_{h}") for h in hs}
                        for hc in range(NHC):
                            for h in hs:
                                nc.tensor.matmul(
                                    ps[h][:],
                                    w_sb[:, hc, h * HD:(h + 1) * HD],
                                    x_sb[:, hc, :],
                                    start=(hc == 0), stop=(hc == NHC - 1))
                        for h in hs:
                            nc.scalar.copy(out_a[:, h, js], ps[h][:])
                            rope(out_a, h, js)

            # V pass: re-read x, pos-major, 8 banks, DVE copies
            wv_sb = wpool.tile([128, NHC, HPC * HD], bf16, tag="w", name="w_v")
            nc.sync.dma_start(out=wv_sb[:], in_=wr["v"][:])
            for j in range(NQC):
                js = slice(j * QC, (j + 1) * QC)
                x_sb = xp.tile([128, NHC, QC], bf16, tag="x", name=f"xv{j}")
                xr = xT[:, js].rearrange("(a p) m -> p a m", p=128)
                for g in range(4):
                    nc.sync.dma_start(out=x_sb[:, 4 * g:4 * g + 4, :],
                                      in_=xr[:, 4 * g:4 * g + 4, :])
                for wv in range(2):
                    pbs = (2 * wv, 2 * wv + 1)
                    psd = {(pb, dc): psA.tile([128, QC], fp32, tag="psA",
                                              name=f"psv{j}_{pb}_{dc}")
                           for pb in pbs for dc in range(2)}
                    for hc in range(NHC):
                        for pb in pbs:
                            for dc in range(2):
                                nc.tensor.matmul(
                                    psd[(pb, dc)][:],
                                    x_sb[:, hc, pb * 128:(pb + 1) * 128],
                                    wv_sb[:, hc, dc * QC:(dc + 1) * QC],
                                    start=(hc == 0), stop=(hc == NHC - 1))
                    for pb in pbs:
                        for dc in range(2):
                            nc.vector.tensor_copy(
                                Va[:, j * 4 + pb, dc * QC:(dc + 1) * QC],
                                psd[(pb, dc)][:])

            tp_cm.__exit__(None, None, None)
            xp_cm.__exit__(None, None, None)
            ropec_cm.__exit__(None, None, None)
            wpool_cm.__exit__(None, None, None)
            psA_cm.__exit__(None, None, None)

            # ---------------- phases B + C ----------------
            with tc.tile_pool(name="post", bufs=1, side="left") as post:
                OTa = post.tile([HD, HPC, L], bf16, tag="ota")
                wo_sb = post.tile([128, HPC, H], bf16, tag="wo")
                nc.sync.dma_start(
                    out=wo_sb[:],
                    in_=woT[:].rearrange("(a p) m -> p a m", p=128))

                _attention(tc, nc, kind, block_pat, QTa, KTa, Va, OTa,
                           cst, fp32, bf16, EXP)

                with tc.tile_pool(name="ysb", bufs=3, side="right") as ypool, \
                     tc.tile_pool(name="ps_c", bufs=4, space="PSUM") as ps_c:
                    for j in range(NQC):
                        for oc in range(H // 128):
                            ps = ps_c.tile([128, QC], fp32, tag="psc")
                            for fc in range(HPC):
                                nc.tensor.matmul(
                                    ps[:],
                                    wo_sb[:, fc, oc * 128:(oc + 1) * 128],
                                    OTa[:, fc, j * QC:(j + 1) * QC],
                                    start=(fc == 0), stop=(fc == HPC - 1))
                            y_sb = ypool.tile([128, QC], bf16, tag="y")
                            nc.vector.tensor_copy(y_sb[:], ps[:])
                            nc.sync.dma_start(
                                out=yT[oc * 128:(oc + 1) * 128,
                                       j * QC:(j + 1) * QC],
                                in_=y_sb[:])

    nc.compile()
    return nc


def _attention(tc, nc, kind, block_pat, QTa, KTa, Va, OTa, cst,
               fp32, bf16, EXP):
    """v1-style jpair attention: q-chunk PAIRS inside the kp-block loop so
    S (and O, rowsum) matmuls sit back-to-back with a shared stationary
    operand; one kp-block of lookahead keeps PE ahead of the ACT exp."""
    with tc.tile_pool(name="pp", bufs=6, side="right") as ppool, \
         tc.tile_pool(name="rr", bufs=4, side="right") as rpool, \
         tc.tile_pool(name="bb", bufs=4, side="right") as bpool, \
         tc.tile_pool(name="sump", bufs=3, side="right") as sump, \
         tc.tile_pool(name="on", bufs=1, side="right") as onp, \
         tc.tile_pool(name="ps_s", bufs=4, space="PSUM") as ps_s, \
         tc.tile_pool(name="ps_o", bufs=1, space="PSUM") as ps_o, \
         tc.tile_pool(name="ps_r", bufs=1, space="PSUM") as ps_r:
        ones_sb = onp.tile([128, 1], bf16, tag="onesb")
        nc.vector.memset(ones_sb[:], 1.0)
        for h in range(HPC):
            for jpair in ((0, 1), (2, 3)):
                blocks_j = {}
                first_i = {}
                last_i = {}
                for j in jpair:
                    for i in range(NKB):
                        live = [t for t in range(4)
                                if kind[4 * j + t][i] != 0]
                        if live:
                            blocks_j.setdefault(i, []).append((j, live))
                            if j not in first_i:
                                first_i[j] = i
                            last_i[j] = i
                if not first_i:
                    continue
                pso = {j: ps_o.tile([128, QC], fp32, tag=f"pso{j % 2}",
                                    name=f"pso{h}_{j}")
                       for j in first_i}
                psr = {j: ps_r.tile([1, QC], fp32, tag=f"psr{j % 2}",
                                    name=f"psr{h}_{j}")
                       for j in first_i}
                rs_pend = {}
                rs_started = set()

                def emit_s(i, j, live):
                    t0, t1 = live[0], live[-1]
                    w0, w1 = t0 * 128, (t1 + 1) * 128
                    pss = ps_s.tile([128, QC], fp32, tag="pss",
                                    name=f"pss{h}_{j}_{i}")
                    nc.tensor.matmul(
                        pss[:, w0:w1],
                        KTa[:, h, i * 128:(i + 1) * 128],
                        QTa[:, h, j * QC + w0:j * QC + w1],
                        start=True, stop=True)
                    P = ppool.tile([128, QC], bf16, tag="p",
                                   name=f"p{h}_{j}_{i}")
                    first = (first_i[j] == i)
                    if w0 > 0 and first:
                        nc.vector.memset(P[:, 0:w0], 0.0)
                    if w1 < QC and first:
                        nc.vector.memset(P[:, w1:QC], 0.0)
                    nc.scalar.activation(P[:, w0:w1], pss[:, w0:w1],
                                         EXP, scale=SCALE)
                    for t in range(t0, t1 + 1):
                        qb = 4 * j + t
                        if kind[qb][i] == 0:
                            nc.vector.memset(
                                P[:, t * 128:(t + 1) * 128], 0.0)
                        elif kind[qb][i] == 2:
                            pat = block_pat[(qb, i)]
                            nc.vector.tensor_mul(
                                P[:, t * 128:(t + 1) * 128],
                                P[:, t * 128:(t + 1) * 128],
                                cst[:, pat * 128:(pat + 1) * 128])
                    return (j, P, w0, first)

                def emit_ovr(i, group):
                    for j, P, w0, first in group:
                        m0 = 0 if first else w0
                        nc.tensor.matmul(
                            pso[j][:, m0:QC],
                            Va[:, i, h * HD:(h + 1) * HD],
                            P[:, m0:QC],
                            start=first, stop=(last_i[j] == i))
                    for j, P, w0, first in group:
                        m0 = 0 if first else w0
                        last = (last_i[j] == i)
                        pend = rs_pend.pop(j, None)
                        if pend is None and not last:
                            rs_pend[j] = (P, m0)
                            continue
                        st = j not in rs_started
                        rs_started.add(j)
                        if pend is None:
                            nc.tensor.matmul(
                                psr[j][0:1, m0:QC], ones_sb[:, 0:1],
                                P[:, m0:QC], start=st, stop=last)
                        else:
                            Pa, m0a = pend
                            Ps = sump.tile([128, QC], bf16, tag="ps2",
                                           name=f"ps2_{h}_{j}_{i}")
                            nc.vector.tensor_add(Ps[:, m0:QC],
                                                 Pa[:, m0:QC], P[:, m0:QC])
                            if m0 > m0a:
                                nc.vector.tensor_copy(Ps[:, m0a:m0],
                                                      Pa[:, m0a:m0])
                            nc.tensor.matmul(
                                psr[j][0:1, m0a:QC], ones_sb[:, 0:1],
                                Ps[:, m0a:QC], start=st, stop=last)
                    for j, P, w0, first in group:
                        if last_i[j] != i:
                            continue
                        r_sb = rpool.tile([128, QC], fp32, tag="r",
                                          name=f"r{h}_{j}")
                        nc.vector.reciprocal_approx_fast(
                            out=r_sb[0:1, :], in_=psr[j][0:1, :])
                        rb_sb = rpool.tile([128, QC], bf16, tag="rb",
                                           name=f"rb{h}_{j}")
                        nc.vector.tensor_copy(rb_sb[0:1, :],
                                              r_sb[0:1, :])
                        bc_sb = bpool.tile([128, QC], bf16, tag="bc",
                                           name=f"bc{h}_{j}")
                        nc.gpsimd.partition_broadcast(bc_sb[:],
                                                      rb_sb[0:1, :])
                        nc.vector.tensor_mul(
                            OTa[:, h, j * QC:(j + 1) * QC],
                            pso[j][:], bc_sb[:])

                prev = None
                for i in sorted(blocks_j):
                    cur = (i, [emit_s(i, j, live)
                               for j, live in blocks_j[i]])
                    if prev is not None:
                        emit_ovr(*prev)
                    prev = cur
                if prev is not None:
                    emit_ovr(*prev)


def _prep_inputs(x, Wq, Wk, Wv, Wo, patterns):
    import ml_dtypes
    bf16 = ml_dtypes.bfloat16

    inv_freq = 1.0 / (ROPE_BASE ** (np.arange(0, HD, 2, dtype=np.float64)
                                    / HD))
    t = np.arange(L, dtype=np.float64)
    freqs = np.outer(t, inv_freq)
    emb = np.concatenate((freqs, freqs), axis=-1)
    cos = np.cos(emb).T.astype(np.float32)
    sin = np.sin(emb).T.astype(np.float32)
    sinm = sin.copy()
    sinm[0:64] = -sin[0:64]
    cos_b = cos.astype(bf16)
    sinm_b = sinm.astype(bf16)

    npat = max(len(patterns), 1)
    maskd = np.zeros((npat, 128, 128), dtype=bf16)
    for i, p in enumerate(patterns):
        maskd[i] = p.astype(np.float32).astype(bf16)

    in_maps = []
    for c in range(NCORES):
        b, half = c // 2, c % 2
        rows = slice(half * HPC * HD, (half + 1) * HPC * HD)
        in_maps.append({
            "xT": np.ascontiguousarray(x[b].T).astype(bf16),
            "wqT": np.ascontiguousarray(Wq[rows, :].T).astype(bf16),
            "wkT": np.ascontiguousarray(Wk[rows, :].T).astype(bf16),
            "wvT": np.ascontiguousarray(Wv[rows, :].T).astype(bf16),
            "woT": np.ascontiguousarray(Wo[:, rows].T).astype(bf16),
            "cosd": cos_b,
            "sinmd": sinm_b,
            "maskd": maskd,
        })
    return in_maps


def kernel(x, mask, Wq, Wk, Wv, Wo, _trace=False):
    from concourse.bass_utils import run_bass_kernel_spmd

    x = np.asarray(x, dtype=np.float32)
    mask2d = np.asarray(mask, dtype=np.int32).reshape(L, L)
    key = mask2d.tobytes()
    if key not in _cache:
        kind, patterns, block_pat = _analyze_mask(mask2d)
        nc = _build(kind, block_pat, len(patterns))
        _cache[key] = (nc, patterns)
    nc, patterns = _cache[key]

    in_maps = _prep_inputs(x, np.asarray(Wq, np.float32),
                           np.asarray(Wk, np.float32),
                           np.asarray(Wv, np.float32),
                           np.asarray(Wo, np.float32), patterns)
    res = run_bass_kernel_spmd(nc, in_maps, list(range(NCORES)),
                               trace=_trace)
    y = np.empty((B, L, H), dtype=np.float32)
    for b in range(B):
        acc = res.results[2 * b]["yT"].astype(np.float32) + \
              res.results[2 * b + 1]["yT"].astype(np.float32)
        y[b] = acc.T
    if _trace:
        kernel.last_results = res
    return y


if __name__ == "__main__":
    import reference
    inputs = reference.setup_inputs()
    inputs = {k: np.asarray(v) for k, v in inputs.items()}
    out = kernel(**inputs)
    exp = np.asarray(reference.reference(**{k: v for k, v in inputs.items()}))
    err = np.abs(out - exp).max() / np.abs(exp).max()
    print("rel err (absmax):", err)


# revision 13
# speedup vs baseline: 1.1354x; 1.0065x over previous
"""Trainium2 Bass kernel: causal multi-head attention with RoPE.

Model: B=4, L=2048, H=2048, NH=16 heads, head_dim=128.
  q = x @ Wq.T ; k = x @ Wk.T ; v = x @ Wv.T        (per-head split)
  q, k <- RoPE(q, k)
  attn = softmax(mask(q k^T / sqrt(hd)))
  out  = (attn @ v) heads-concat @ Wo.T

Sharding (8 cores): hybrid batch x tensor-parallel.  Core c handles
batch b = c//2 and heads half*8..half*8+7 with half = c%2.  Wq/Wk/Wv are
column-sharded (8 heads per core), Wo row-sharded; each core produces a
partial y[b] (bf16) and the host sums the two partials per batch.

Per-core dataflow (SBUF-resident, bf16 inputs / fp32 accumulation):
  phase A: QK pass per 512-pos x chunk (x loaded once for both):
           hc-outer accumulation into 8 PSUM banks so the first matmul
           needs only 1.5MB of DMA; fused RoPE (partition-swap DMA +
           DVE).  Then a V pass (pos-major, 8 banks) with PSUM->SBUF
           copies on DVE so ACT is drained before attention.
  phase B: flash-style causal attention per (head, 512-wide q chunk):
           S^T tiles into a 5-bank PSUM ring, exp on ACT over multi-bank
           group spans (group sizes DP-chosen to trade ACT instruction
           overhead vs dead-margin columns), triangular-mask multiply on
           diagonal blocks (DVE), O^T accumulation (PE), softmax
           denominator via fp8 DoubleRow ones-matmul over pair-packed
           fp8 copies of P (DVE converts), per-head batched
           normalization (unnormalized O commutes with the denominator).
  phase C: y^T partial = Wo_shard O^T (PE) -> DRAM bf16.
"""

import math
import numpy as np

B, L, H, NH, HD = 4, 2048, 2048, 16, 128
ROPE_BASE = 10000.0
NCORES = 8
HPC = 8          # heads per core
QC = 512         # q chunk width
NQC = L // QC    # 4 q chunks
NKB = L // 128   # 16 kp blocks
SCALE = 1.0 / math.sqrt(HD)
NHC = H // 128   # 16 input-feature blocks

_cache = {}


def _analyze_mask(mask2d):
    """Classify each (q_block, kp_block) 128x128 block of the [L, L] mask."""
    nb = L // 128
    kind = [[0] * nb for _ in range(nb)]
    patterns = []
    pat_key_to_idx = {}
    block_pat = {}
    for qb in range(nb):
        rows = mask2d[qb * 128:(qb + 1) * 128]
        for kb in range(nb):
            blk = rows[:, kb * 128:(kb + 1) * 128]
            s = int(blk.sum())
            if s == 0:
                kind[qb][kb] = 0
            elif s == 128 * 128:
                kind[qb][kb] = 1
            else:
                kind[qb][kb] = 2
                key = blk.tobytes()
                idx = pat_key_to_idx.get(key)
                if idx is None:
                    idx = len(patterns)
                    pat_key_to_idx[key] = idx
                    # stored transposed: S^T tiles are [kp, q]
                    patterns.append(np.ascontiguousarray(blk.T))
                block_pat[(qb, kb)] = idx
    return kind, patterns, block_pat


def _chunk_plan(kind, block_pat):
    """Per q-chunk block list: (i, w0, [(t, pat), ...]) for live kp blocks.

    w0 = first live 128-col offset within the chunk; requires the causal
    staircase (w0 nondecreasing in i, first block full, even count)."""
    plans = []
    for j in range(NQC):
        blocks = []
        for i in range(NKB):
            live = [t for t in range(4) if kind[4 * j + t][i] != 0]
            if not live:
                continue
            w0 = live[0] * 128
            assert live == list(range(live[0], 4)), "non-staircase mask"
            diags = [(t, block_pat[(4 * j + t, i)]) for t in live
                     if kind[4 * j + t][i] == 2]
            blocks.append((i, w0, diags))
        assert blocks and blocks[0][1] == 0, "first live block must be full"
        assert len(blocks) % 2 == 0, "need even live-block count per chunk"
        for a, b in zip(blocks, blocks[1:]):
            assert a[1] <= b[1], "w0 must be nondecreasing"
        plans.append(blocks)
    return plans


def _plan_groups(blocks, parity0):
    """Split a chunk's blocks into exp groups for the 5-slot PSUM ring.

    Ring = A slots (0,1) + B slots (2,3,4), strictly alternating; a group
    may underfill.  DP minimizes ACT cost = sum(OH + len*512 - w0_first).
    Returns (groups, parity_out), groups = lists of indices into blocks."""
    OH = 390
    n = len(blocks)
    INF = float("inf")
    dp = [[None, None] for _ in range(n + 1)]
    dp[n] = [(0, 0), (0, 0)]
    for i in range(n - 1, -1, -1):
        for p in (0, 1):
            cap = 2
            best = (INF, 0)
            for ln in range(1, min(cap, n - i) + 1):
                cost = OH + ln * 512 - blocks[i][1] + dp[i + ln][1 - p][0]
                if cost < best[0]:
                    best = (cost, ln)
            dp[i][p] = best
    groups = []
    i, p = 0, parity0
    while i < n:
        ln = dp[i][p][1]
        groups.append(list(range(i, i + ln)))
        i += ln
        p = 1 - p
    return groups, p


def _build(kind, block_pat, n_patterns):
    """Build the SPMD bass program (same for all 8 cores)."""
    import concourse.bacc as bacc
    import concourse.mybir as mybir
    import concourse.tile as tile
    from concourse.tile import add_dep_helper

    fp32 = mybir.dt.float32
    bf16 = mybir.dt.bfloat16
    fp8 = mybir.dt.float8e4
    EXP = mybir.ActivationFunctionType.Exp
    DR = mybir.MatmulPerfMode.DoubleRow

    nc = bacc.Bacc("TRN2", target_bir_lowering=False, debug=False)

    xT = nc.dram_tensor("xT", [H, L], bf16, kind="ExternalInput")
    wqT = nc.dram_tensor("wqT", [H, HPC * HD], bf16, kind="ExternalInput")
    wkT = nc.dram_tensor("wkT", [H, HPC * HD], bf16, kind="ExternalInput")
    wvT = nc.dram_tensor("wvT", [H, HPC * HD], bf16, kind="ExternalInput")
    woT = nc.dram_tensor("woT", [HPC * HD, H], bf16, kind="ExternalInput")
    cosd = nc.dram_tensor("cosd", [HD, L], bf16, kind="ExternalInput")
    sinmd = nc.dram_tensor("sinmd", [HD, L], bf16, kind="ExternalInput")
    npat = max(n_patterns, 1)
    maskd = nc.dram_tensor("maskd", [npat, 128, 128], bf16, kind="ExternalInput")
    yT = nc.dram_tensor("yT", [H, L], bf16, kind="ExternalOutput")

    plans = _chunk_plan(kind, block_pat)

    with tile.TileContext(nc) as tc:
        with tc.tile_pool(name="persist", bufs=1, side="left") as persist:
            cst = persist.tile([128, npat * 128], bf16, tag="cst")
            ones8 = persist.tile([128, 2, 16], fp8, tag="ones8")
            QTa = persist.tile([HD, HPC, L], bf16, tag="qta")
            KTa = persist.tile([HD, HPC, L], bf16, tag="kta")
            Va = persist.tile([128, NKB, HPC * HD], bf16, tag="va")

            # ---------------- phase A ----------------
            wpool_cm = tc.tile_pool(name="wpool", bufs=2, side="right")
            wpool = wpool_cm.__enter__()
            ropec_cm = tc.tile_pool(name="ropec", bufs=1, side="right")
            ropec = ropec_cm.__enter__()
            xp_cm = tc.tile_pool(name="xp", bufs=2, side="right")
            xp = xp_cm.__enter__()
            tp_cm = tc.tile_pool(name="tpool", bufs=2, side="right")
            tp = tp_cm.__enter__()
            psA_cm = tc.tile_pool(name="psA", bufs=8, space="PSUM")
            psA = psA_cm.__enter__()

            wq_sb = wpool.tile([128, NHC, HPC * HD], bf16, tag="w", name="w_q")
            wk_sb = wpool.tile([128, NHC, HPC * HD], bf16, tag="w", name="w_k")
            cos_sb = ropec.tile([HD, L], bf16, tag="cos")
            sinm_sb = ropec.tile([HD, L], bf16, tag="sinm")

            wr = {"q": wqT[:].rearrange("(a p) m -> p a m", p=128),
                  "k": wkT[:].rearrange("(a p) m -> p a m", p=128),
                  "v": wvT[:].rearrange("(a p) m -> p a m", p=128)}

            # startup: interleave x(j0) and wq groups; defer the rest
            x0_sb = xp.tile([128, NHC, QC], bf16, tag="x", name="x0")
            xr0 = xT[:, 0:QC].rearrange("(a p) m -> p a m", p=128)
            x0_dmas, wq_dmas = [], []
            for g in range(8):
                wq_dmas.append(nc.sync.dma_start(
                    out=wq_sb[:, 2 * g:2 * g + 2, :],
                    in_=wr["q"][:, 2 * g:2 * g + 2, :]))
                x0_dmas.append(nc.sync.dma_start(
                    out=x0_sb[:, 2 * g:2 * g + 2, :],
                    in_=xr0[:, 2 * g:2 * g + 2, :]))
            nc.gpsimd.dma_start(out=cos_sb[:], in_=cosd[:])
            nc.gpsimd.dma_start(out=sinm_sb[:], in_=sinmd[:])
            for p in range(n_patterns):
                nc.gpsimd.dma_start(out=cst[:, p * 128:(p + 1) * 128],
                                    in_=maskd[p])
            nc.vector.memset(ones8[:], 1.0)
            for g in range(4):
                di = nc.sync.dma_start(
                    out=wk_sb[:, 4 * g:4 * g + 4, :],
                    in_=wr["k"][:, 4 * g:4 * g + 4, :])
                for d0 in (x0_dmas[7], wq_dmas[7]):
                    add_dep_helper(di.ins, d0.ins, reason="defer wk")

            def rope(out_a, h, js):
                q = out_a[:, h, js]
                rq = tp.tile([128, QC], bf16, tag="rot")
                nc.sync.dma_start(out=rq[0:64, :], in_=out_a[64:128, h, js])
                nc.sync.dma_start(out=rq[64:128, :], in_=out_a[0:64, h, js])
                nc.vector.tensor_mul(rq[:], rq[:], sinm_sb[:, js])
                nc.vector.tensor_mul(q, q, cos_sb[:, js])
                nc.vector.tensor_add(q, q, rq[:])

            # QK pass: x chunk loaded once, Q then K, hc-outer, 8 banks
            for j in range(NQC):
                js = slice(j * QC, (j + 1) * QC)
                if j == 0:
                    x_sb = x0_sb
                else:
                    x_sb = xp.tile([128, NHC, QC], bf16, tag="x", name=f"x{j}")
                    xr = xT[:, js].rearrange("(a p) m -> p a m", p=128)
                    for g in range(4):
                        nc.sync.dma_start(out=x_sb[:, 4 * g:4 * g + 4, :],
                                          in_=xr[:, 4 * g:4 * g + 4, :])
                for w_sb, out_a, tag in ((wq_sb, QTa, "q"), (wk_sb, KTa, "k")):
                    for wv in range(2):
                        hs = range(4 * wv, 4 * wv + 4)
                        ps = {h: psA.tile([128, QC], fp32, tag="psA",
                                          name=f"ps{tag}{j}_{h}") for h in hs}
                        for hc in range(NHC):
                            for h in hs:
                                nc.tensor.matmul(
                                    ps[h][:],
                                    w_sb[:, hc, h * HD:(h + 1) * HD],
                                    x_sb[:, hc, :],
                                    start=(hc == 0), stop=(hc == NHC - 1))
                        for h in hs:
                            nc.scalar.copy(out_a[:, h, js], ps[h][:])
                            rope(out_a, h, js)

            # V pass: re-read x, pos-major, 8 banks, DVE copies
            wv_sb = wpool.tile([128, NHC, HPC * HD], bf16, tag="w", name="w_v")
            nc.sync.dma_start(out=wv_sb[:], in_=wr["v"][:])
            for j in range(NQC):
                js = slice(j * QC, (j + 1) * QC)
                x_sb = xp.tile([128, NHC, QC], bf16, tag="x", name=f"xv{j}")
                xr = xT[:, js].rearrange("(a p) m -> p a m", p=128)
                for g in range(4):
                    nc.sync.dma_start(out=x_sb[:, 4 * g:4 * g + 4, :],
                                      in_=xr[:, 4 * g:4 * g + 4, :])
                for wv in range(2):
                    pbs = (2 * wv, 2 * wv + 1)
                    psd = {(pb, dc): psA.tile([128, QC], fp32, tag="psA",
                                              name=f"psv{j}_{pb}_{dc}")
                           for pb in pbs for dc in range(2)}
                    for hc in range(NHC):
                        for pb in pbs:
                            for dc in range(2):
                                nc.tensor.matmul(
                                    psd[(pb, dc)][:],
                                    x_sb[:, hc, pb * 128:(pb + 1) * 128],
                                    wv_sb[:, hc, dc * QC:(dc + 1) * QC],
                                    start=(hc == 0), stop=(hc == NHC - 1))
                    for pb in pbs:
                        for dc in range(2):
                            dst = Va[:, j * 4 + pb, dc * QC:(dc + 1) * QC]
                            if j == NQC - 1 and (pb + dc) % 2 == 1:
                                nc.scalar.copy(dst, psd[(pb, dc)][:])
                            else:
                                nc.vector.tensor_copy(dst, psd[(pb, dc)][:])

            tp_cm.__exit__(None, None, None)
            xp_cm.__exit__(None, None, None)
            ropec_cm.__exit__(None, None, None)
            wpool_cm.__exit__(None, None, None)
            psA_cm.__exit__(None, None, None)

            # ---------------- phases B + C ----------------
            with tc.tile_pool(name="post", bufs=1, side="left") as post:
                OTa = post.tile([HD, HPC, L], bf16, tag="ota")
                wo_sb = post.tile([128, HPC, H], bf16, tag="wo")
                nc.sync.dma_start(
                    out=wo_sb[:],
                    in_=woT[:].rearrange("(a p) m -> p a m", p=128))

                _attention(tc, nc, kind, block_pat, QTa, KTa, Va, OTa,
                           cst, fp32, bf16, EXP)

                with tc.tile_pool(name="ysb", bufs=3, side="right") as ypool, \
                     tc.tile_pool(name="ps_c", bufs=4, space="PSUM") as ps_c:
                    for j in range(NQC):
                        for oc in range(H // 128):
                            ps = ps_c.tile([128, QC], fp32, tag="psc")
                            for fc in range(HPC):
                                nc.tensor.matmul(
                                    ps[:],
                                    wo_sb[:, fc, oc * 128:(oc + 1) * 128],
                                    OTa[:, fc, j * QC:(j + 1) * QC],
                                    start=(fc == 0), stop=(fc == HPC - 1))
                            y_sb = ypool.tile([128, QC], bf16, tag="y")
                            nc.vector.tensor_copy(y_sb[:], ps[:])
                            nc.sync.dma_start(
                                out=yT[oc * 128:(oc + 1) * 128,
                                       j * QC:(j + 1) * QC],
                                in_=y_sb[:])

    nc.compile()
    return nc


def _attention(tc, nc, kind, block_pat, QTa, KTa, Va, OTa, cst,
               fp32, bf16, EXP):
    """v1-style jpair attention: q-chunk PAIRS inside the kp-block loop so
    S (and O, rowsum) matmuls sit back-to-back with a shared stationary
    operand; one kp-block of lookahead keeps PE ahead of the ACT exp."""
    with tc.tile_pool(name="pp", bufs=6, side="right") as ppool, \
         tc.tile_pool(name="rr", bufs=4, side="right") as rpool, \
         tc.tile_pool(name="bb", bufs=4, side="right") as bpool, \
         tc.tile_pool(name="sump", bufs=3, side="right") as sump, \
         tc.tile_pool(name="on", bufs=1, side="right") as onp, \
         tc.tile_pool(name="ps_s", bufs=4, space="PSUM") as ps_s, \
         tc.tile_pool(name="ps_o", bufs=1, space="PSUM") as ps_o, \
         tc.tile_pool(name="ps_r", bufs=1, space="PSUM") as ps_r:
        ones_sb = onp.tile([128, 1], bf16, tag="onesb")
        nc.vector.memset(ones_sb[:], 1.0)
        for h in range(HPC):
            for jpair in ((0, 1), (2, 3)):
                blocks_j = {}
                first_i = {}
                last_i = {}
                for j in jpair:
                    for i in range(NKB):
                        live = [t for t in range(4)
                                if kind[4 * j + t][i] != 0]
                        if live:
                            blocks_j.setdefault(i, []).append((j, live))
                            if j not in first_i:
                                first_i[j] = i
                            last_i[j] = i
                if not first_i:
                    continue
                pso = {j: ps_o.tile([128, QC], fp32, tag=f"pso{j % 2}",
                                    name=f"pso{h}_{j}")
                       for j in first_i}
                psr = {j: ps_r.tile([1, QC], fp32, tag=f"psr{j % 2}",
                                    name=f"psr{h}_{j}")
                       for j in first_i}
                rs_pend = {}
                rs_started = set()

                def emit_s(i, j, live):
                    t0, t1 = live[0], live[-1]
                    w0, w1 = t0 * 128, (t1 + 1) * 128
                    pss = ps_s.tile([128, QC], fp32, tag="pss",
                                    name=f"pss{h}_{j}_{i}")
                    nc.tensor.matmul(
                        pss[:, w0:w1],
                        KTa[:, h, i * 128:(i + 1) * 128],
                        QTa[:, h, j * QC + w0:j * QC + w1],
                        start=True, stop=True)
                    P = ppool.tile([128, QC], bf16, tag="p",
                                   name=f"p{h}_{j}_{i}")
                    first = (first_i[j] == i)
                    if w0 > 0 and first:
                        nc.vector.memset(P[:, 0:w0], 0.0)
                    if w1 < QC and first:
                        nc.vector.memset(P[:, w1:QC], 0.0)
                    nc.scalar.activation(P[:, w0:w1], pss[:, w0:w1],
                                         EXP, scale=SCALE)
                    for t in range(t0, t1 + 1):
                        qb = 4 * j + t
                        if kind[qb][i] == 0:
                            nc.vector.memset(
                                P[:, t * 128:(t + 1) * 128], 0.0)
                        elif kind[qb][i] == 2:
                            pat = block_pat[(qb, i)]
                            nc.vector.tensor_mul(
                                P[:, t * 128:(t + 1) * 128],
                                P[:, t * 128:(t + 1) * 128],
                                cst[:, pat * 128:(pat + 1) * 128])
                    return (j, P, w0, first)

                def emit_ovr(i, group):
                    for j, P, w0, first in group:
                        m0 = 0 if first else w0
                        nc.tensor.matmul(
                            pso[j][:, m0:QC],
                            Va[:, i, h * HD:(h + 1) * HD],
                            P[:, m0:QC],
                            start=first, stop=(last_i[j] == i))
                    for j, P, w0, first in group:
                        m0 = 0 if first else w0
                        last = (last_i[j] == i)
                        pend = rs_pend.setdefault(j, [])
                        pend.append((P, m0))
                        if len(pend) < 3 and not last:
                            continue
                        rs_pend[j] = []
                        st = j not in rs_started
                        rs_started.add(j)
                        m00 = pend[0][1]
                        if len(pend) == 1:
                            src_ap = pend[0][0][:, m00:QC]
                        else:
                            Ps = sump.tile([128, QC], bf16, tag="ps2",
                                           name=f"ps2_{h}_{j}_{i}")
                            (Pa, m0a), (Pb, m0b) = pend[0], pend[1]
                            nc.vector.tensor_add(Ps[:, m0b:QC],
                                                 Pa[:, m0b:QC],
                                                 Pb[:, m0b:QC])
                            if m0b > m0a:
                                nc.vector.tensor_copy(Ps[:, m0a:m0b],
                                                      Pa[:, m0a:m0b])
                            for Pc, m0c in pend[2:]:
                                nc.vector.tensor_add(Ps[:, m0c:QC],
                                                     Ps[:, m0c:QC],
                                                     Pc[:, m0c:QC])
                            src_ap = Ps[:, m00:QC]
                        nc.tensor.matmul(
                            psr[j][0:1, m00:QC], ones_sb[:, 0:1],
                            src_ap, start=st, stop=last)
                    for j, P, w0, first in group:
                        if last_i[j] != i:
                            continue
                        r_sb = rpool.tile([128, QC], fp32, tag="r",
                                          name=f"r{h}_{j}")
                        nc.vector.reciprocal_approx_fast(
                            out=r_sb[0:1, :], in_=psr[j][0:1, :])
                        rb_sb = rpool.tile([128, QC], bf16, tag="rb",
                                           name=f"rb{h}_{j}")
                        nc.vector.tensor_copy(rb_sb[0:1, :],
                                              r_sb[0:1, :])
                        bc_sb = bpool.tile([128, QC], bf16, tag="bc",
                                           name=f"bc{h}_{j}")
                        nc.gpsimd.partition_broadcast(bc_sb[:],
                                                      rb_sb[0:1, :])
                        nc.vector.tensor_mul(
                            OTa[:, h, j * QC:(j + 1) * QC],
                            pso[j][:], bc_sb[:])

                prev = None
                for i in sorted(blocks_j):
                    cur = (i, [emit_s(i, j, live)
                               for j, live in blocks_j[i]])
                    if prev is not None:
                        emit_ovr(*prev)
                    prev = cur
                if prev is not None:
                    emit_ovr(*prev)


def _prep_inputs(x, Wq, Wk, Wv, Wo, patterns):
    import ml_dtypes
    bf16 = ml_dtypes.bfloat16

    inv_freq = 1.0 / (ROPE_BASE ** (np.arange(0, HD, 2, dtype=np.float64)
                                    / HD))
    t = np.arange(L, dtype=np.float64)
    freqs = np.outer(t, inv_freq)
    emb = np.concatenate((freqs, freqs), axis=-1)
    cos = np.cos(emb).T.astype(np.float32)
    sin = np.sin(emb).T.astype(np.float32)
    sinm = sin.copy()
    sinm[0:64] = -sin[0:64]
    cos_b = cos.astype(bf16)
    sinm_b = sinm.astype(bf16)

    npat = max(len(patterns), 1)
    maskd = np.zeros((npat, 128, 128), dtype=bf16)
    for i, p in enumerate(patterns):
        maskd[i] = p.astype(np.float32).astype(bf16)

    in_maps = []
    for c in range(NCORES):
        b, half = c // 2, c % 2
        rows = slice(half * HPC * HD, (half + 1) * HPC * HD)
        in_maps.append({
            "xT": np.ascontiguousarray(x[b].T).astype(bf16),
            "wqT": np.ascontiguousarray(Wq[rows, :].T).astype(bf16),
            "wkT": np.ascontiguousarray(Wk[rows, :].T).astype(bf16),
            "wvT": np.ascontiguousarray(Wv[rows, :].T).astype(bf16),
            "woT": np.ascontiguousarray(Wo[:, rows].T).astype(bf16),
            "cosd": cos_b,
            "sinmd": sinm_b,
            "maskd": maskd,
        })
    return in_maps


def kernel(x, mask, Wq, Wk, Wv, Wo, _trace=False):
    from concourse.bass_utils import run_bass_kernel_spmd

    x = np.asarray(x, dtype=np.float32)
    mask2d = np.asarray(mask, dtype=np.int32).reshape(L, L)
    key = mask2d.tobytes()
    if key not in _cache:
        kind, patterns, block_pat = _analyze_mask(mask2d)
        nc = _build(kind, block_pat, len(patterns))
        _cache[key] = (nc, patterns)
    nc, patterns = _cache[key]

    in_maps = _prep_inputs(x, np.asarray(Wq, np.float32),
                           np.asarray(Wk, np.float32),
                           np.asarray(Wv, np.float32),
                           np.asarray(Wo, np.float32), patterns)
    res = run_bass_kernel_spmd(nc, in_maps, list(range(NCORES)),
                               trace=_trace)
    y = np.empty((B, L, H), dtype=np.float32)
    for b in range(B):
        acc = res.results[2 * b]["yT"].astype(np.float32) + \
              res.results[2 * b + 1]["yT"].astype(np.float32)
        y[b] = acc.T
    if _trace:
        kernel.last_results = res
    return y


if __name__ == "__main__":
    import reference
    inputs = reference.setup_inputs()
    inputs = {k: np.asarray(v) for k, v in inputs.items()}
    out = kernel(**inputs)
    exp = np.asarray(reference.reference(**{k: v for k, v in inputs.items()}))
    err = np.abs(out - exp).max() / np.abs(exp).max()
    print("rel err (absmax):", err)
